# revision 31
# baseline (speedup 1.0000x reference)
"""NodeConv kernel for 8 Trainium2 NeuronCores.

Reference computes, for adj [B,1,N,N], node [B,nin,N], Wi/Wj [nout,nin]:
    x  = node[:, :, None, :] * adj          # [B,nin,N,N]
    yi = einsum('oc,bcij->boij', Wi, x)
    yj = einsum('oc,bcij->boij', Wj, x)
    out = I * yi + (1-I) * yj

Because adj[b,i,j] does not depend on the contraction channel c, the
contraction factors out:
    off-diag: out[b,o,i,j] = adj[b,i,j] * (Wj @ node[b])[o,j]
    diag:     out[b,o,j,j] = adj[b,j,j] * (Wi @ node[b])[o,j]

Sharding: core c handles batch b=c//2, row half h=c%2 (128 rows). Odd
halves get their columns rolled by -128 on the host so the diagonal of
local row l sits at local column l on every core -> one SPMD program;
the host rolls the output back while gathering.

Design (measured down from the 72.8us f32 baseline to ~54us; tolerance
is 2e-2 and this path measures ~7.4e-3 max rel err):
  - OUTPUT IS BF16 (host upconverts to f32 while gathering): halves the
    HBM store traffic 16 MiB -> 8 MiB per core.  DMA floor ~21us.
  - adj is split host-side into 4 scaled e4m3 terms (reconstruction
    error ~4e-6) stored on psum partitions 0-3, chunk-major along the
    free dim, with a constant [4, NOUT] stationary holding the descale
    factors (1, 2^-4, 2^-8, 2^-8).  Each 512-col fp8 matmul broadcasts
    adj to all 128 output partitions in one pass (PE streams 1 col/cycle
    at the 1.2 GHz mid p-state regardless of dtype, so K and dtype are
    chosen purely to minimize the input load: 128 KB).
  - weights/node ship as bf16 so the u = Wj@node and dv = Wi@nodeD
    matmuls are single-pass (fp32 PE matmuls run as 2 half-speed
    passes).  nodeD is the diagonal node columns prescaled by adj's
    diagonal on the host, so dv IS the diagonal patch value - no
    diag-broadcast matmul needed.
  - per chunk (4 rows x 256 = 1024 cols, [128,1024] f32 psum tiles,
    4-buffer rotation so the PE->consumer round-trip never idles the
    pipeline): A-chunks are multiplied straight from PSUM on DVE
    (f32 x bf16 -> bf16, 1 elem/lane/cycle); B-chunks are copied
    PSUM f32 -> bf16 SBUF by ScalarE, then DVE multiplies all-bf16 in
    2x_1P mode (2 elem/lane/cycle).  The A/B mix balances ScalarE
    (~26us) against DVE (~27us) - the body floor.
  - diagonal patches (strided [128, RCH] writes of dv) run on the
    otherwise-idle GpSimd engine.
  - stores are issued from the Sync and GpSimd queues, group sizes
    tapered (small first groups start the DMA flow early, small last
    groups shorten the final drain); NODECONV_GORDER permutes group
    processing order.
  - GSET chunks (off by default) use GpSimd partition_broadcast of a
    host-rounded bf16 adj instead of the PE path; measured slower
    (~4.1us per chunk at 0.41 efficiency) but kept as a knob.
"""

import os

import numpy as np

NCORES = 8
B, N, NIN, NOUT = 4, 256, 128, 128
RPC = 128          # rows per core
RCH = int(os.environ.get("NODECONV_RCH", "4"))   # rows per chunk
CH = RPC // RCH    # chunks per core
FREE = RCH * N     # free elems per chunk
PSUM_BUFS = int(os.environ.get("NODECONV_PSUM_BUFS", "4" if RCH <= 4 else "2"))

# G-chunks: GpSimd partition_broadcast of host-rounded bf16 adj + DVE 2x_1P
# multiply — no PE/PSUM/ScalarE involvement.  Placed at the edges so the
# first stores fire early and the tail chunks are compute-ready early.
_GSET = {
    int(x)
    for x in os.environ.get("NODECONV_GSET", "").split(",")
    if x != ""
}
# C-chunks: ScalarE bf16 staging copy + GpSimd (standard-library) multiply —
# relieves the saturated DVE using GpSimd idle time.  Patches for these run
# in-order on GpSimd right after the multiply (no cross-engine semaphore).
_CSET = {
    int(x)
    for x in os.environ.get(
        "NODECONV_CSET", ""
    ).split(",")
    if x != ""
}
# A-chunks: multiplied directly from PSUM on DVE (1x); remaining (B) chunks
# go through a ScalarE bf16 staging copy + DVE 2x_1P multiply
_ASET = {
    int(x)
    for x in os.environ.get(
        "NODECONV_ASET",
        "0,4,8,12,16,20,24,27,30,31" if RCH == 4 else "0,5,10,15",
    ).split(",")
    if x != ""
}
_G = [
    int(x)
    for x in os.environ.get(
        "NODECONV_GROUPS",
        "1,1,2,2,4,4,4,4,4,2,2,2" if RCH == 4 else "2,2,2,2,2,2,2,1,1",
    ).split(",")
]
assert sum(_G) == CH
# processing order of the groups: DRAM-tail groups run early so the final
# store is not serialized behind the final DRAM addresses; the
# last-processed group is small for a short drain
_GORDER = [
    int(x)
    for x in os.environ.get(
        "NODECONV_GORDER",
        "0,1,10,11,2,3,4,5,6,7,8,9" if RCH == 4 else ",".join(map(str, range(9))),
    ).split(",")
]
assert sorted(_GORDER) == list(range(len(_G)))
STAGE_BUFS = int(os.environ.get("NODECONV_STAGE_BUFS", "6"))
OUT_BUFS = int(os.environ.get("NODECONV_OUT_BUFS", "6"))
PATCH_ENG = os.environ.get("NODECONV_PATCH", "gp")  # gp | scalar | vector

KP = 4             # fp8 term partitions (t0..t3)

_cached = {}

last_results = None  # BassKernelResults of the most recent kernel() call


def _build_nc():
    key = (
        RCH,
        PSUM_BUFS,
        tuple(sorted(_ASET)),
        tuple(sorted(_CSET)),
        tuple(sorted(_GSET)),
        tuple(_G),
        tuple(_GORDER),
        STAGE_BUFS,
        OUT_BUFS,
        PATCH_ENG,
    )
    if key in _cached:
        return _cached[key]

    from contextlib import ExitStack

    import concourse.tile as tile
    from concourse import bacc, mybir

    f32 = mybir.dt.float32
    bf16 = mybir.dt.bfloat16
    fp8 = mybir.dt.float8e4

    nc = bacc.Bacc(
        "TRN2", target_bir_lowering=False, debug=False, num_devices=NCORES
    )

    # pk8: [4, CH*FREE + NOUT] fp8 — partition t holds scaled adj term t for
    # every chunk (chunk-major along the free dim), then the [4, NOUT]
    # stationary selector carrying the descale factors (1, 2^-4, 2^-8,
    # 2^-8).  Every matmul slices this at base partition 0 with the SAME
    # stationary, so no per-chunk selector blocks are needed.
    pk8 = nc.dram_tensor(
        "pk8", [KP, CH * FREE + NOUT], fp8, kind="ExternalInput"
    ).ap()
    # ckf: [128, 640] bf16 — node_r | WiT | WjT | nodeD, where nodeD is the
    # diagonal node columns prescaled by adj's diagonal on the host, so
    # Wi @ nodeD directly yields the diagonal patch values dv.  bf16 keeps
    # the u/dv matmuls single-pass (fp32 PE matmuls run as 2 half-speed
    # passes) and halves the critical first input load.
    ckf = nc.dram_tensor(
        "ckf", [NIN, N + 2 * NOUT + RPC], bf16, kind="ExternalInput"
    ).ap()
    # af: chunk p's eight adj rows flattened on partition 0 (bf16) — the
    # source for GpSimd partition_broadcast on G-chunks
    af = nc.dram_tensor("af", [1, RPC * N], bf16, kind="ExternalInput").ap()
    out = nc.dram_tensor("out", [NOUT, RPC * N], bf16, kind="ExternalOutput").ap()

    with tile.TileContext(nc) as tc, ExitStack() as ctx:
        const = ctx.enter_context(tc.tile_pool(name="const", bufs=1))
        psum = ctx.enter_context(tc.tile_pool(name="psum", bufs=PSUM_BUFS, space="PSUM"))
        outp = ctx.enter_context(tc.tile_pool(name="outp", bufs=OUT_BUFS))
        stage = ctx.enter_context(tc.tile_pool(name="stage", bufs=STAGE_BUFS))
        bcp = ctx.enter_context(tc.tile_pool(name="bcp", bufs=len(_GSET) or 1))

        # Both loads on the sync queue (it issues earliest), ckf first: it is
        # smaller and heads the u-chain that every multiply depends on.
        ckf_sb = const.tile([NIN, N + 2 * NOUT + RPC], bf16)
        nc.sync.dma_start(out=ckf_sb[:], in_=ckf)
        pk8_sb = const.tile([KP, CH * FREE + NOUT], fp8)
        nc.sync.dma_start(out=pk8_sb[:], in_=pk8)
        af_sb = const.tile([1, RPC * N], bf16)
        nc.gpsimd.dma_start(out=af_sb[:], in_=af)

        node_sb = ckf_sb[:, 0:N]
        wit_sb = ckf_sb[:, N : N + NOUT]
        wjt_sb = ckf_sb[:, N + NOUT : N + 2 * NOUT]
        noded_sb = ckf_sb[:, N + 2 * NOUT : N + 2 * NOUT + RPC]
        sel_v = pk8_sb[:, CH * FREE : CH * FREE + NOUT]
        pk_v = pk8_sb[:, 0 : CH * FREE]

        # u = Wj @ node_r -> [nout, N], and (later) dv = Wi @ nodeD.  Both
        # live in one psum tile so the chunk psum tiles keep alternating
        # between the pool's two buffers.
        ps_uv = psum.tile([NOUT, N + RPC], f32, tag="mm")
        nc.tensor.matmul(
            ps_uv[:, 0:N], lhsT=wjt_sb, rhs=node_sb, start=True, stop=True
        )
        u_bf = const.tile([NOUT, N], bf16)
        nc.scalar.copy(u_bf[:], ps_uv[:, 0:N])
        # dv = Wi @ nodeD — must be written before the first patch reads it
        dv_bf = const.tile([NOUT, RPC], bf16)
        nc.tensor.matmul(
            ps_uv[:, N : N + RPC], lhsT=wit_sb, rhs=noded_sb, start=True, stop=True
        )
        nc.scalar.copy(dv_bf[:], ps_uv[:, N : N + RPC])

        # u replicated RCH (and 2*RCH for fused B-pairs) times along the
        # free dim via stride-0 views
        u16_rep = u_bf[:].unsqueeze(1).broadcast_to([NOUT, RCH, N])
        u16_rep2 = u_bf[:].unsqueeze(1).broadcast_to([NOUT, 2 * RCH, N])

        patch_eng = {
            "gp": nc.gpsimd,
            "scalar": nc.scalar,
            "vector": nc.vector,
        }[PATCH_ENG]

        gstart = [0]
        for gsz in _G:
            gstart.append(gstart[-1] + gsz)
        for oi, gi in enumerate(_GORDER):
            gsz = _G[gi]
            o_sb = outp.tile([NOUT, gsz * FREE], bf16, tag="osb")
            p0 = gstart[gi]
            # adjacent B-chunks within a group share one stage tile and a
            # single fused DVE multiply over both (fewer DVE instructions)
            bpair = {}
            run = []
            for g in range(gsz):
                q_ = p0 + g
                if q_ not in _ASET and q_ not in _GSET and q_ not in _CSET:
                    run.append(g)
                    if len(run) == 2:
                        st2 = stage.tile([NOUT, 2 * FREE], bf16, tag="st")
                        bpair[run[0]] = (st2, 0, None)
                        bpair[run[1]] = (st2, 1, run[0])
                        run = []
                else:
                    run = []
            def emit_patch(eng_, g_, p_):
                # diagonal of local row l sits at free offset RCH*p + k*257
                eng_.tensor_scalar_mul(
                    o_sb[
                        :,
                        g_ * FREE + RCH * p_ : g_ * FREE
                        + RCH * p_
                        + (RCH - 1) * (N + 1)
                        + 1 : N + 1,
                    ],
                    dv_bf[:, RCH * p_ : RCH * (p_ + 1)],
                    1.0,
                )

            for g in range(gsz):
                p = p0 + g
                defer_patch = False
                o_view = o_sb[:, g * FREE : (g + 1) * FREE].rearrange(
                    "p (k j) -> p k j", k=RCH
                )
                if p in _GSET:
                    bc = bcp.tile([NOUT, FREE], bf16, tag="bc")
                    nc.gpsimd.partition_broadcast(
                        bc[:], af_sb[:, FREE * p : FREE * (p + 1)]
                    )
                    nc.vector.tensor_mul(
                        o_view, bc[:].rearrange("p (k j) -> p k j", k=RCH), u16_rep
                    )
                    peng = nc.vector
                else:
                    ps_b = psum.tile([NOUT, FREE], f32, tag="mm")
                    for q in range(FREE // 512):
                        nc.tensor.matmul(
                            ps_b[:, 512 * q : 512 * (q + 1)],
                            lhsT=sel_v,
                            rhs=pk_v[
                                :, FREE * p + 512 * q : FREE * p + 512 * (q + 1)
                            ],
                            start=True,
                            stop=True,
                        )
                    if p in _ASET:
                        nc.vector.tensor_mul(
                            o_view,
                            ps_b[:].rearrange("p (k j) -> p k j", k=RCH),
                            u16_rep,
                        )
                    elif p in _CSET:
                        st = stage.tile([NOUT, FREE], bf16, tag="st")
                        nc.scalar.copy(st[:], ps_b[:])
                        nc.gpsimd.tensor_mul(
                            o_view,
                            st[:].rearrange("p (k j) -> p k j", k=RCH),
                            u16_rep,
                        )
                    elif g in bpair:
                        st2, half, first_g = bpair[g]
                        nc.scalar.copy(
                            st2[:, half * FREE : (half + 1) * FREE], ps_b[:]
                        )
                        if half == 1:
                            # both halves staged: one fused 2-chunk multiply,
                            # then the deferred patch of the pair's first
                            # chunk (it must come after the fused multiply)
                            nc.vector.tensor_mul(
                                o_sb[
                                    :, first_g * FREE : (first_g + 2) * FREE
                                ].rearrange("p (k j) -> p k j", k=2 * RCH),
                                st2[:].rearrange("p (k j) -> p k j", k=2 * RCH),
                                u16_rep2,
                            )
                            emit_patch(patch_eng, first_g, p0 + first_g)
                        else:
                            defer_patch = True
                    else:
                        st = stage.tile([NOUT, FREE], bf16, tag="st")
                        nc.scalar.copy(st[:], ps_b[:])
                        nc.vector.tensor_mul(
                            o_view,
                            st[:].rearrange("p (k j) -> p k j", k=RCH),
                            u16_rep,
                        )
                    peng = patch_eng
                if not defer_patch:
                    emit_patch(peng, g, p)
            eng = nc.sync if oi % 2 == 0 else nc.gpsimd
            eng.dma_start(
                out=out[:, FREE * p0 : FREE * (p0 + gsz)], in_=o_sb[:]
            )

    nc.compile()
    _cached[key] = nc
    return nc


def _split_fp8_terms(x):
    """Split fp32 array (values in [0,1)) into 4 e4m3 terms with scales
    (1, 2^4, 2^8, 2^8) whose descaled f32 sum reconstructs x to ~4e-6."""
    import ml_dtypes

    f8 = ml_dtypes.float8_e4m3
    t0 = x.astype(f8)
    r = x - t0.astype(np.float32)
    t1 = (r * 16.0).astype(f8)
    r = r - t1.astype(np.float32) / 16.0
    t2 = (r * 256.0).astype(f8)
    r = r - t2.astype(np.float32) / 256.0
    t3 = (r * 256.0).astype(f8)
    return t0, t1, t2, t3


def _in_maps(adj, node, Wi, Wj):
    import ml_dtypes

    f8 = ml_dtypes.float8_e4m3
    sel = np.empty((KP, NOUT), f8)
    for t, s in enumerate([1.0, 2.0**-4, 2.0**-8, 2.0**-8]):
        sel[t, :] = s
    bf = ml_dtypes.bfloat16
    ckf = np.empty((NIN, N + 2 * NOUT + RPC), bf)
    ckf[:, N : N + NOUT] = Wi.T
    ckf[:, N + NOUT : N + 2 * NOUT] = Wj.T
    bf = ml_dtypes.bfloat16
    maps = []
    for c in range(NCORES):
        b, h = divmod(c, 2)
        r0 = RPC * h
        a = adj[b, 0, r0 : r0 + RPC, :]
        if h:
            ar = np.roll(a, -r0, axis=1)
            noder = np.roll(node[b], -r0, axis=1)
        else:
            ar = a
            noder = node[b]
        t0, t1, t2, t3 = _split_fp8_terms(ar.reshape(1, RPC * N))
        pk8 = np.empty((KP, CH * FREE + NOUT), f8)
        pk8[0, 0 : CH * FREE] = t0[0]
        pk8[1, 0 : CH * FREE] = t1[0]
        pk8[2, 0 : CH * FREE] = t2[0]
        pk8[3, 0 : CH * FREE] = t3[0]
        pk8[:, CH * FREE :] = sel
        m_ckf = ckf.copy()
        m_ckf[:, 0:N] = noder
        adiag = a[np.arange(RPC), r0 + np.arange(RPC)]
        m_ckf[:, N + 2 * NOUT :] = noder[:, 0:RPC] * adiag[None, :]
        af = ar.reshape(1, RPC * N).astype(bf)
        maps.append({"pk8": pk8, "ckf": m_ckf, "af": af})
    return maps


def kernel(**inputs):
    global last_results
    adj = np.asarray(inputs["adj"], dtype=np.float32)
    node = np.asarray(inputs["node"], dtype=np.float32)
    Wi = np.asarray(inputs["Wi"], dtype=np.float32)
    Wj = np.asarray(inputs["Wj"], dtype=np.float32)

    from concourse.bass_utils import run_bass_kernel_spmd

    nc = _build_nc()
    res = run_bass_kernel_spmd(nc, _in_maps(adj, node, Wi, Wj), list(range(NCORES)))
    last_results = res

    out = np.empty((B, NOUT, N, N), np.float32)
    for c in range(NCORES):
        b, h = divmod(c, 2)
        co = res.results[c]["out"].astype(np.float32).reshape(NOUT, RPC, N)
        if h:
            co = np.roll(co, RPC * h, axis=2)
        out[b, :, RPC * h : RPC * (h + 1), :] = co
    return out


# revision 32
# speedup vs baseline: 1.0074x; 1.0074x over previous
"""NodeConv kernel for 8 Trainium2 NeuronCores.

Reference computes, for adj [B,1,N,N], node [B,nin,N], Wi/Wj [nout,nin]:
    x  = node[:, :, None, :] * adj          # [B,nin,N,N]
    yi = einsum('oc,bcij->boij', Wi, x)
    yj = einsum('oc,bcij->boij', Wj, x)
    out = I * yi + (1-I) * yj

Because adj[b,i,j] does not depend on the contraction channel c, the
contraction factors out:
    off-diag: out[b,o,i,j] = adj[b,i,j] * (Wj @ node[b])[o,j]
    diag:     out[b,o,j,j] = adj[b,j,j] * (Wi @ node[b])[o,j]

Sharding: core c handles batch b=c//2, row half h=c%2 (128 rows). Odd
halves get their columns rolled by -128 on the host so the diagonal of
local row l sits at local column l on every core -> one SPMD program;
the host rolls the output back while gathering.

Design (measured down from the 72.8us f32 baseline to ~54us; tolerance
is 2e-2 and this path measures ~7.4e-3 max rel err):
  - OUTPUT IS BF16 (host upconverts to f32 while gathering): halves the
    HBM store traffic 16 MiB -> 8 MiB per core.  DMA floor ~21us.
  - adj is split host-side into 4 scaled e4m3 terms (reconstruction
    error ~4e-6) stored on psum partitions 0-3, chunk-major along the
    free dim, with a constant [4, NOUT] stationary holding the descale
    factors (1, 2^-4, 2^-8, 2^-8).  Each 512-col fp8 matmul broadcasts
    adj to all 128 output partitions in one pass (PE streams 1 col/cycle
    at the 1.2 GHz mid p-state regardless of dtype, so K and dtype are
    chosen purely to minimize the input load: 128 KB).
  - weights/node ship as bf16 so the u = Wj@node and dv = Wi@nodeD
    matmuls are single-pass (fp32 PE matmuls run as 2 half-speed
    passes).  nodeD is the diagonal node columns prescaled by adj's
    diagonal on the host, so dv IS the diagonal patch value - no
    diag-broadcast matmul needed.
  - per chunk (4 rows x 256 = 1024 cols, [128,1024] f32 psum tiles,
    4-buffer rotation so the PE->consumer round-trip never idles the
    pipeline): A-chunks are multiplied straight from PSUM on DVE
    (f32 x bf16 -> bf16, 1 elem/lane/cycle); B-chunks are copied
    PSUM f32 -> bf16 SBUF by ScalarE, then DVE multiplies all-bf16 in
    2x_1P mode (2 elem/lane/cycle).  The A/B mix balances ScalarE
    (~26us) against DVE (~27us) - the body floor.
  - diagonal patches (strided [128, RCH] writes of dv) run on the
    otherwise-idle GpSimd engine.
  - stores are issued from the Sync and GpSimd queues, group sizes
    tapered (small first groups start the DMA flow early, small last
    groups shorten the final drain); NODECONV_GORDER permutes group
    processing order.
  - GSET chunks (off by default) use GpSimd partition_broadcast of a
    host-rounded bf16 adj instead of the PE path; measured slower
    (~4.1us per chunk at 0.41 efficiency) but kept as a knob.
"""

import os

import numpy as np

NCORES = 8
B, N, NIN, NOUT = 4, 256, 128, 128
RPC = 128          # rows per core
RCH = int(os.environ.get("NODECONV_RCH", "4"))   # rows per chunk
CH = RPC // RCH    # chunks per core
FREE = RCH * N     # free elems per chunk
PSUM_BUFS = int(os.environ.get("NODECONV_PSUM_BUFS", "4" if RCH <= 4 else "2"))

# G-chunks: GpSimd partition_broadcast of host-rounded bf16 adj + DVE 2x_1P
# multiply — no PE/PSUM/ScalarE involvement.  Placed at the edges so the
# first stores fire early and the tail chunks are compute-ready early.
_GSET = {
    int(x)
    for x in os.environ.get("NODECONV_GSET", "").split(",")
    if x != ""
}
# C-chunks: ScalarE bf16 staging copy + GpSimd (standard-library) multiply —
# relieves the saturated DVE using GpSimd idle time.  Patches for these run
# in-order on GpSimd right after the multiply (no cross-engine semaphore).
_CSET = {
    int(x)
    for x in os.environ.get(
        "NODECONV_CSET", ""
    ).split(",")
    if x != ""
}
# A-chunks: multiplied directly from PSUM on DVE (1x); remaining (B) chunks
# go through a ScalarE bf16 staging copy + DVE 2x_1P multiply
_ASET = {
    int(x)
    for x in os.environ.get(
        "NODECONV_ASET",
        "0,4,8,12,16,20,24,27,30,31" if RCH == 4 else "0,5,10,15",
    ).split(",")
    if x != ""
}
_G = [
    int(x)
    for x in os.environ.get(
        "NODECONV_GROUPS",
        "1,1,2,2,4,4,4,4,4,2,2,2" if RCH == 4 else "2,2,2,2,2,2,2,1,1",
    ).split(",")
]
assert sum(_G) == CH
# processing order of the groups: DRAM-tail groups run early so the final
# store is not serialized behind the final DRAM addresses; the
# last-processed group is small for a short drain
_GORDER = [
    int(x)
    for x in os.environ.get(
        "NODECONV_GORDER",
        "0,1,10,11,2,3,4,5,6,7,8,9" if RCH == 4 else ",".join(map(str, range(9))),
    ).split(",")
]
assert sorted(_GORDER) == list(range(len(_G)))
STAGE_BUFS = int(os.environ.get("NODECONV_STAGE_BUFS", "6"))
OUT_BUFS = int(os.environ.get("NODECONV_OUT_BUFS", "6"))
PATCH_ENG = os.environ.get("NODECONV_PATCH", "gp")  # gp | scalar | vector

KP = 4             # fp8 term partitions (t0..t3)

_cached = {}

last_results = None  # BassKernelResults of the most recent kernel() call


def _build_nc():
    key = (
        RCH,
        PSUM_BUFS,
        tuple(sorted(_ASET)),
        tuple(sorted(_CSET)),
        tuple(sorted(_GSET)),
        tuple(_G),
        tuple(_GORDER),
        STAGE_BUFS,
        OUT_BUFS,
        PATCH_ENG,
    )
    if key in _cached:
        return _cached[key]

    from contextlib import ExitStack

    import concourse.tile as tile
    from concourse import bacc, mybir

    f32 = mybir.dt.float32
    bf16 = mybir.dt.bfloat16
    fp8 = mybir.dt.float8e4

    nc = bacc.Bacc(
        "TRN2", target_bir_lowering=False, debug=False, num_devices=NCORES
    )

    # pk8: [4, CH*FREE + NOUT] fp8 — partition t holds scaled adj term t for
    # every chunk (chunk-major along the free dim), then the [4, NOUT]
    # stationary selector carrying the descale factors (1, 2^-4, 2^-8,
    # 2^-8).  Every matmul slices this at base partition 0 with the SAME
    # stationary, so no per-chunk selector blocks are needed.
    pk8 = nc.dram_tensor(
        "pk8", [KP, CH * FREE + NOUT], fp8, kind="ExternalInput"
    ).ap()
    # ckf: [128, 640] bf16 — node_r | WiT | WjT | nodeD, where nodeD is the
    # diagonal node columns prescaled by adj's diagonal on the host, so
    # Wi @ nodeD directly yields the diagonal patch values dv.  bf16 keeps
    # the u/dv matmuls single-pass (fp32 PE matmuls run as 2 half-speed
    # passes) and halves the critical first input load.
    ckf = nc.dram_tensor(
        "ckf", [NIN, N + 2 * NOUT + RPC], bf16, kind="ExternalInput"
    ).ap()
    # af: chunk p's eight adj rows flattened on partition 0 (bf16) — the
    # source for GpSimd partition_broadcast on G-chunks
    af = nc.dram_tensor("af", [1, RPC * N], bf16, kind="ExternalInput").ap()
    out = nc.dram_tensor("out", [NOUT, RPC * N], bf16, kind="ExternalOutput").ap()

    with tile.TileContext(nc) as tc, ExitStack() as ctx:
        const = ctx.enter_context(tc.tile_pool(name="const", bufs=1))
        psum = ctx.enter_context(tc.tile_pool(name="psum", bufs=PSUM_BUFS, space="PSUM"))
        outp = ctx.enter_context(tc.tile_pool(name="outp", bufs=OUT_BUFS))
        stage = ctx.enter_context(tc.tile_pool(name="stage", bufs=STAGE_BUFS))
        bcp = ctx.enter_context(tc.tile_pool(name="bcp", bufs=len(_GSET) or 1))

        # Input loads are split across the sync and gpsimd queues: the
        # 4-partition pk8 tensor transfers slowly per descriptor, so each
        # queue carries half, and ckf's u-critical piece (node|WjT) plus the
        # matmul selector go out first on sync.
        ckf_sb = const.tile([NIN, N + 2 * NOUT + RPC], bf16)
        pk8_sb = const.tile([KP, CH * FREE + NOUT], fp8)
        half = (CH // 2) * FREE
        nc.sync.dma_start(out=ckf_sb[:, 0 : N + NOUT], in_=ckf[:, 0 : N + NOUT])
        nc.sync.dma_start(
            out=pk8_sb[:, CH * FREE :], in_=pk8[:, CH * FREE :]
        )
        nc.sync.dma_start(out=pk8_sb[:, 0:half], in_=pk8[:, 0:half])
        nc.gpsimd.dma_start(
            out=ckf_sb[:, N + NOUT :], in_=ckf[:, N + NOUT :]
        )
        nc.gpsimd.dma_start(
            out=pk8_sb[:, half : CH * FREE], in_=pk8[:, half : CH * FREE]
        )
        if _GSET:
            af_sb = const.tile([1, RPC * N], bf16)
            nc.gpsimd.dma_start(out=af_sb[:], in_=af)

        node_sb = ckf_sb[:, 0:N]
        wjt_sb = ckf_sb[:, N : N + NOUT]
        wit_sb = ckf_sb[:, N + NOUT : N + 2 * NOUT]
        noded_sb = ckf_sb[:, N + 2 * NOUT : N + 2 * NOUT + RPC]
        sel_v = pk8_sb[:, CH * FREE : CH * FREE + NOUT]
        pk_v = pk8_sb[:, 0 : CH * FREE]

        # u = Wj @ node_r -> [nout, N], and (later) dv = Wi @ nodeD.  Both
        # live in one psum tile so the chunk psum tiles keep alternating
        # between the pool's two buffers.
        ps_uv = psum.tile([NOUT, N + RPC], f32, tag="mm")
        nc.tensor.matmul(
            ps_uv[:, 0:N], lhsT=wjt_sb, rhs=node_sb, start=True, stop=True
        )
        u_bf = const.tile([NOUT, N], bf16)
        nc.scalar.copy(u_bf[:], ps_uv[:, 0:N])
        # dv = Wi @ nodeD — must be written before the first patch reads it
        dv_bf = const.tile([NOUT, RPC], bf16)
        nc.tensor.matmul(
            ps_uv[:, N : N + RPC], lhsT=wit_sb, rhs=noded_sb, start=True, stop=True
        )
        nc.scalar.copy(dv_bf[:], ps_uv[:, N : N + RPC])

        # u replicated RCH (and 2*RCH for fused B-pairs) times along the
        # free dim via stride-0 views
        u16_rep = u_bf[:].unsqueeze(1).broadcast_to([NOUT, RCH, N])
        u16_rep2 = u_bf[:].unsqueeze(1).broadcast_to([NOUT, 2 * RCH, N])

        patch_eng = {
            "gp": nc.gpsimd,
            "scalar": nc.scalar,
            "vector": nc.vector,
        }[PATCH_ENG]

        gstart = [0]
        for gsz in _G:
            gstart.append(gstart[-1] + gsz)
        for oi, gi in enumerate(_GORDER):
            gsz = _G[gi]
            o_sb = outp.tile([NOUT, gsz * FREE], bf16, tag="osb")
            p0 = gstart[gi]
            # adjacent B-chunks within a group share one stage tile and a
            # single fused DVE multiply over both (fewer DVE instructions)
            bpair = {}
            run = []
            for g in range(gsz):
                q_ = p0 + g
                if q_ not in _ASET and q_ not in _GSET and q_ not in _CSET:
                    run.append(g)
                    if len(run) == 2:
                        st2 = stage.tile([NOUT, 2 * FREE], bf16, tag="st")
                        bpair[run[0]] = (st2, 0, None)
                        bpair[run[1]] = (st2, 1, run[0])
                        run = []
                else:
                    run = []
            def emit_patch(eng_, g_, p_):
                # diagonal of local row l sits at free offset RCH*p + k*257
                eng_.tensor_scalar_mul(
                    o_sb[
                        :,
                        g_ * FREE + RCH * p_ : g_ * FREE
                        + RCH * p_
                        + (RCH - 1) * (N + 1)
                        + 1 : N + 1,
                    ],
                    dv_bf[:, RCH * p_ : RCH * (p_ + 1)],
                    1.0,
                )

            for g in range(gsz):
                p = p0 + g
                defer_patch = False
                o_view = o_sb[:, g * FREE : (g + 1) * FREE].rearrange(
                    "p (k j) -> p k j", k=RCH
                )
                if p in _GSET:
                    bc = bcp.tile([NOUT, FREE], bf16, tag="bc")
                    nc.gpsimd.partition_broadcast(
                        bc[:], af_sb[:, FREE * p : FREE * (p + 1)]
                    )
                    nc.vector.tensor_mul(
                        o_view, bc[:].rearrange("p (k j) -> p k j", k=RCH), u16_rep
                    )
                    peng = nc.vector
                else:
                    ps_b = psum.tile([NOUT, FREE], f32, tag="mm")
                    for q in range(FREE // 512):
                        nc.tensor.matmul(
                            ps_b[:, 512 * q : 512 * (q + 1)],
                            lhsT=sel_v,
                            rhs=pk_v[
                                :, FREE * p + 512 * q : FREE * p + 512 * (q + 1)
                            ],
                            start=True,
                            stop=True,
                        )
                    if p in _ASET:
                        nc.vector.tensor_mul(
                            o_view,
                            ps_b[:].rearrange("p (k j) -> p k j", k=RCH),
                            u16_rep,
                        )
                    elif p in _CSET:
                        st = stage.tile([NOUT, FREE], bf16, tag="st")
                        nc.scalar.copy(st[:], ps_b[:])
                        nc.gpsimd.tensor_mul(
                            o_view,
                            st[:].rearrange("p (k j) -> p k j", k=RCH),
                            u16_rep,
                        )
                    elif g in bpair:
                        st2, half, first_g = bpair[g]
                        nc.scalar.copy(
                            st2[:, half * FREE : (half + 1) * FREE], ps_b[:]
                        )
                        if half == 1:
                            # both halves staged: one fused 2-chunk multiply,
                            # then the deferred patch of the pair's first
                            # chunk (it must come after the fused multiply)
                            nc.vector.tensor_mul(
                                o_sb[
                                    :, first_g * FREE : (first_g + 2) * FREE
                                ].rearrange("p (k j) -> p k j", k=2 * RCH),
                                st2[:].rearrange("p (k j) -> p k j", k=2 * RCH),
                                u16_rep2,
                            )
                            emit_patch(patch_eng, first_g, p0 + first_g)
                        else:
                            defer_patch = True
                    else:
                        st = stage.tile([NOUT, FREE], bf16, tag="st")
                        nc.scalar.copy(st[:], ps_b[:])
                        nc.vector.tensor_mul(
                            o_view,
                            st[:].rearrange("p (k j) -> p k j", k=RCH),
                            u16_rep,
                        )
                    peng = patch_eng
                if not defer_patch:
                    emit_patch(peng, g, p)
            eng = nc.sync if oi % 2 == 0 else nc.gpsimd
            eng.dma_start(
                out=out[:, FREE * p0 : FREE * (p0 + gsz)], in_=o_sb[:]
            )

    nc.compile()
    _cached[key] = nc
    return nc


def _split_fp8_terms(x):
    """Split fp32 array (values in [0,1)) into 4 e4m3 terms with scales
    (1, 2^4, 2^8, 2^8) whose descaled f32 sum reconstructs x to ~4e-6."""
    import ml_dtypes

    f8 = ml_dtypes.float8_e4m3
    t0 = x.astype(f8)
    r = x - t0.astype(np.float32)
    t1 = (r * 16.0).astype(f8)
    r = r - t1.astype(np.float32) / 16.0
    t2 = (r * 256.0).astype(f8)
    r = r - t2.astype(np.float32) / 256.0
    t3 = (r * 256.0).astype(f8)
    return t0, t1, t2, t3


def _in_maps(adj, node, Wi, Wj):
    import ml_dtypes

    f8 = ml_dtypes.float8_e4m3
    sel = np.empty((KP, NOUT), f8)
    for t, s in enumerate([1.0, 2.0**-4, 2.0**-8, 2.0**-8]):
        sel[t, :] = s
    bf = ml_dtypes.bfloat16
    ckf = np.empty((NIN, N + 2 * NOUT + RPC), bf)
    ckf[:, N : N + NOUT] = Wj.T
    ckf[:, N + NOUT : N + 2 * NOUT] = Wi.T
    bf = ml_dtypes.bfloat16
    maps = []
    for c in range(NCORES):
        b, h = divmod(c, 2)
        r0 = RPC * h
        a = adj[b, 0, r0 : r0 + RPC, :]
        if h:
            ar = np.roll(a, -r0, axis=1)
            noder = np.roll(node[b], -r0, axis=1)
        else:
            ar = a
            noder = node[b]
        t0, t1, t2, t3 = _split_fp8_terms(ar.reshape(1, RPC * N))
        pk8 = np.empty((KP, CH * FREE + NOUT), f8)
        pk8[0, 0 : CH * FREE] = t0[0]
        pk8[1, 0 : CH * FREE] = t1[0]
        pk8[2, 0 : CH * FREE] = t2[0]
        pk8[3, 0 : CH * FREE] = t3[0]
        pk8[:, CH * FREE :] = sel
        m_ckf = ckf.copy()
        m_ckf[:, 0:N] = noder
        adiag = a[np.arange(RPC), r0 + np.arange(RPC)]
        m_ckf[:, N + 2 * NOUT :] = noder[:, 0:RPC] * adiag[None, :]
        af = ar.reshape(1, RPC * N).astype(bf)
        maps.append({"pk8": pk8, "ckf": m_ckf, "af": af})
    return maps


def kernel(**inputs):
    global last_results
    adj = np.asarray(inputs["adj"], dtype=np.float32)
    node = np.asarray(inputs["node"], dtype=np.float32)
    Wi = np.asarray(inputs["Wi"], dtype=np.float32)
    Wj = np.asarray(inputs["Wj"], dtype=np.float32)

    from concourse.bass_utils import run_bass_kernel_spmd

    nc = _build_nc()
    res = run_bass_kernel_spmd(nc, _in_maps(adj, node, Wi, Wj), list(range(NCORES)))
    last_results = res

    out = np.empty((B, NOUT, N, N), np.float32)
    for c in range(NCORES):
        b, h = divmod(c, 2)
        co = res.results[c]["out"].astype(np.float32).reshape(NOUT, RPC, N)
        if h:
            co = np.roll(co, RPC * h, axis=2)
        out[b, :, RPC * h : RPC * (h + 1), :] = co
    return out


# revision 33
# speedup vs baseline: 1.0327x; 1.0252x over previous
"""NodeConv kernel for 8 Trainium2 NeuronCores.

Reference computes, for adj [B,1,N,N], node [B,nin,N], Wi/Wj [nout,nin]:
    x  = node[:, :, None, :] * adj          # [B,nin,N,N]
    yi = einsum('oc,bcij->boij', Wi, x)
    yj = einsum('oc,bcij->boij', Wj, x)
    out = I * yi + (1-I) * yj

Because adj[b,i,j] does not depend on the contraction channel c, the
contraction factors out:
    off-diag: out[b,o,i,j] = adj[b,i,j] * (Wj @ node[b])[o,j]
    diag:     out[b,o,j,j] = adj[b,j,j] * (Wi @ node[b])[o,j]

Sharding: core c handles batch b=c//2, row half h=c%2 (128 rows). Odd
halves get their columns rolled by -128 on the host so the diagonal of
local row l sits at local column l on every core -> one SPMD program;
the host rolls the output back while gathering.

Design (measured down from the 72.8us f32 baseline to ~54us; tolerance
is 2e-2 and this path measures ~7.4e-3 max rel err):
  - OUTPUT IS BF16 (host upconverts to f32 while gathering): halves the
    HBM store traffic 16 MiB -> 8 MiB per core.  DMA floor ~21us.
  - adj is split host-side into 4 scaled e4m3 terms (reconstruction
    error ~4e-6) stored on psum partitions 0-3, chunk-major along the
    free dim, with a constant [4, NOUT] stationary holding the descale
    factors (1, 2^-4, 2^-8, 2^-8).  Each 512-col fp8 matmul broadcasts
    adj to all 128 output partitions in one pass (PE streams 1 col/cycle
    at the 1.2 GHz mid p-state regardless of dtype, so K and dtype are
    chosen purely to minimize the input load: 128 KB).
  - weights/node ship as bf16 so the u = Wj@node and dv = Wi@nodeD
    matmuls are single-pass (fp32 PE matmuls run as 2 half-speed
    passes).  nodeD is the diagonal node columns prescaled by adj's
    diagonal on the host, so dv IS the diagonal patch value - no
    diag-broadcast matmul needed.
  - per chunk (4 rows x 256 = 1024 cols, [128,1024] f32 psum tiles,
    4-buffer rotation so the PE->consumer round-trip never idles the
    pipeline): A-chunks are multiplied straight from PSUM on DVE
    (f32 x bf16 -> bf16, 1 elem/lane/cycle); B-chunks are copied
    PSUM f32 -> bf16 SBUF by ScalarE, then DVE multiplies all-bf16 in
    2x_1P mode (2 elem/lane/cycle).  The A/B mix balances ScalarE
    (~26us) against DVE (~27us) - the body floor.
  - diagonal patches (strided [128, RCH] writes of dv) run on the
    otherwise-idle GpSimd engine.
  - stores are issued from the Sync and GpSimd queues, group sizes
    tapered (small first groups start the DMA flow early, small last
    groups shorten the final drain); NODECONV_GORDER permutes group
    processing order.
  - GSET chunks (off by default) use GpSimd partition_broadcast of a
    host-rounded bf16 adj instead of the PE path; measured slower
    (~4.1us per chunk at 0.41 efficiency) but kept as a knob.
"""

import os

import numpy as np

NCORES = 8
B, N, NIN, NOUT = 4, 256, 128, 128
RPC = 128          # rows per core
RCH = int(os.environ.get("NODECONV_RCH", "4"))   # rows per chunk
CH = RPC // RCH    # chunks per core
FREE = RCH * N     # free elems per chunk
PSUM_BUFS = int(os.environ.get("NODECONV_PSUM_BUFS", "4" if RCH <= 4 else "2"))

# G-chunks: GpSimd partition_broadcast of host-rounded bf16 adj + DVE 2x_1P
# multiply — no PE/PSUM/ScalarE involvement.  Placed at the edges so the
# first stores fire early and the tail chunks are compute-ready early.
_GSET = {
    int(x)
    for x in os.environ.get("NODECONV_GSET", "").split(",")
    if x != ""
}
# C-chunks: ScalarE bf16 staging copy + GpSimd (standard-library) multiply —
# relieves the saturated DVE using GpSimd idle time.  Patches for these run
# in-order on GpSimd right after the multiply (no cross-engine semaphore).
_CSET = {
    int(x)
    for x in os.environ.get(
        "NODECONV_CSET", ""
    ).split(",")
    if x != ""
}
# A-chunks: multiplied directly from PSUM on DVE (1x); remaining (B) chunks
# go through a ScalarE bf16 staging copy + DVE 2x_1P multiply
_ASET = {
    int(x)
    for x in os.environ.get(
        "NODECONV_ASET",
        "0,4,8,12,16,20,24,27,30,31" if RCH == 4 else "0,5,10,15",
    ).split(",")
    if x != ""
}
_G = [
    int(x)
    for x in os.environ.get(
        "NODECONV_GROUPS",
        "1,1,2,2,4,4,4,4,4,2,2,2" if RCH == 4 else "2,2,2,2,2,2,2,1,1",
    ).split(",")
]
assert sum(_G) == CH
# processing order of the groups: DRAM-tail groups run early so the final
# store is not serialized behind the final DRAM addresses; the
# last-processed group is small for a short drain
_GORDER = [
    int(x)
    for x in os.environ.get(
        "NODECONV_GORDER",
        "0,1,10,11,2,3,4,5,6,7,8,9" if RCH == 4 else ",".join(map(str, range(9))),
    ).split(",")
]
assert sorted(_GORDER) == list(range(len(_G)))
STAGE_BUFS = int(os.environ.get("NODECONV_STAGE_BUFS", "6"))
OUT_BUFS = int(os.environ.get("NODECONV_OUT_BUFS", "6"))
PATCH_ENG = os.environ.get("NODECONV_PATCH", "gp")  # gp | scalar | vector

KP = 4             # fp8 term partitions (t0..t3)

_cached = {}

last_results = None  # BassKernelResults of the most recent kernel() call


def _build_nc():
    key = (
        RCH,
        PSUM_BUFS,
        tuple(sorted(_ASET)),
        tuple(sorted(_CSET)),
        tuple(sorted(_GSET)),
        tuple(_G),
        tuple(_GORDER),
        STAGE_BUFS,
        OUT_BUFS,
        PATCH_ENG,
    )
    if key in _cached:
        return _cached[key]

    from contextlib import ExitStack

    import concourse.tile as tile
    from concourse import bacc, mybir

    f32 = mybir.dt.float32
    bf16 = mybir.dt.bfloat16
    fp8 = mybir.dt.float8e4

    nc = bacc.Bacc(
        "TRN2", target_bir_lowering=False, debug=False, num_devices=NCORES
    )

    # pk8: [4, CH*FREE + NOUT] fp8 — partition t holds scaled adj term t for
    # every chunk (chunk-major along the free dim), then the [4, NOUT]
    # stationary selector carrying the descale factors (1, 2^-4, 2^-8,
    # 2^-8).  Every matmul slices this at base partition 0 with the SAME
    # stationary, so no per-chunk selector blocks are needed.
    pk8 = nc.dram_tensor(
        "pk8", [KP, CH * FREE + NOUT], fp8, kind="ExternalInput"
    ).ap()
    # ckf: [128, 640] bf16 — node_r | WiT | WjT | nodeD, where nodeD is the
    # diagonal node columns prescaled by adj's diagonal on the host, so
    # Wi @ nodeD directly yields the diagonal patch values dv.  bf16 keeps
    # the u/dv matmuls single-pass (fp32 PE matmuls run as 2 half-speed
    # passes) and halves the critical first input load.
    ckf = nc.dram_tensor(
        "ckf", [NIN, N + 2 * NOUT + RPC], bf16, kind="ExternalInput"
    ).ap()
    # af: chunk p's eight adj rows flattened on partition 0 (bf16) — the
    # source for GpSimd partition_broadcast on G-chunks
    af = nc.dram_tensor("af", [1, RPC * N], bf16, kind="ExternalInput").ap()
    out = nc.dram_tensor("out", [NOUT, RPC * N], bf16, kind="ExternalOutput").ap()

    with tile.TileContext(nc) as tc, ExitStack() as ctx:
        const = ctx.enter_context(tc.tile_pool(name="const", bufs=1))
        psum = ctx.enter_context(tc.tile_pool(name="psum", bufs=PSUM_BUFS, space="PSUM"))
        outp = ctx.enter_context(tc.tile_pool(name="outp", bufs=OUT_BUFS))
        stage = ctx.enter_context(tc.tile_pool(name="stage", bufs=STAGE_BUFS))
        bcp = ctx.enter_context(tc.tile_pool(name="bcp", bufs=len(_GSET) or 1))

        # Input loads are split across the sync and gpsimd queues: the
        # 4-partition pk8 tensor transfers slowly per descriptor, so each
        # queue carries half, and ckf's u-critical piece (node|WjT) plus the
        # matmul selector go out first on sync.
        ckf_sb = const.tile([NIN, N + 2 * NOUT + RPC], bf16)
        pk8_sb = const.tile([KP, CH * FREE + NOUT], fp8)
        half = (CH // 2) * FREE
        nc.sync.dma_start(out=pk8_sb[:, 0:half], in_=pk8[:, 0:half])
        nc.sync.dma_start(
            out=pk8_sb[:, CH * FREE :], in_=pk8[:, CH * FREE :]
        )
        nc.sync.dma_start(out=ckf_sb[:, 0 : N + NOUT], in_=ckf[:, 0 : N + NOUT])
        nc.gpsimd.dma_start(
            out=pk8_sb[:, half : CH * FREE], in_=pk8[:, half : CH * FREE]
        )
        nc.gpsimd.dma_start(
            out=ckf_sb[:, N + NOUT :], in_=ckf[:, N + NOUT :]
        )
        if _GSET:
            af_sb = const.tile([1, RPC * N], bf16)
            nc.gpsimd.dma_start(out=af_sb[:], in_=af)

        node_sb = ckf_sb[:, 0:N]
        wjt_sb = ckf_sb[:, N : N + NOUT]
        wit_sb = ckf_sb[:, N + NOUT : N + 2 * NOUT]
        noded_sb = ckf_sb[:, N + 2 * NOUT : N + 2 * NOUT + RPC]
        sel_v = pk8_sb[:, CH * FREE : CH * FREE + NOUT]
        pk_v = pk8_sb[:, 0 : CH * FREE]

        # u = Wj @ node_r -> [nout, N], and (later) dv = Wi @ nodeD.  Both
        # live in one psum tile so the chunk psum tiles keep alternating
        # between the pool's two buffers.
        u_bf = const.tile([NOUT, N], bf16)
        dv_bf = const.tile([NOUT, RPC], bf16)
        _uv_emitted = [False]

        def emit_uv():
            # u = Wj @ node_r and dv = Wi @ nodeD, emitted right after the
            # first chunk's matmuls so the pk8-gated broadcast work heads
            # the PE queue; the dv copy precedes every patch in program
            # order (patches read dv_bf)
            ps_uv = psum.tile([NOUT, N + RPC], f32, tag="mm")
            nc.tensor.matmul(
                ps_uv[:, 0:N], lhsT=wjt_sb, rhs=node_sb, start=True, stop=True
            )
            nc.scalar.copy(u_bf[:], ps_uv[:, 0:N])
            nc.tensor.matmul(
                ps_uv[:, N : N + RPC],
                lhsT=wit_sb,
                rhs=noded_sb,
                start=True,
                stop=True,
            )
            nc.scalar.copy(dv_bf[:], ps_uv[:, N : N + RPC])
            _uv_emitted[0] = True

        # u replicated RCH (and 2*RCH for fused B-pairs) times along the
        # free dim via stride-0 views
        u16_rep = u_bf[:].unsqueeze(1).broadcast_to([NOUT, RCH, N])
        u16_rep2 = u_bf[:].unsqueeze(1).broadcast_to([NOUT, 2 * RCH, N])

        patch_eng = {
            "gp": nc.gpsimd,
            "scalar": nc.scalar,
            "vector": nc.vector,
        }[PATCH_ENG]

        gstart = [0]
        for gsz in _G:
            gstart.append(gstart[-1] + gsz)
        for oi, gi in enumerate(_GORDER):
            gsz = _G[gi]
            o_sb = outp.tile([NOUT, gsz * FREE], bf16, tag="osb")
            p0 = gstart[gi]
            # adjacent B-chunks within a group share one stage tile and a
            # single fused DVE multiply over both (fewer DVE instructions)
            bpair = {}
            run = []
            for g in range(gsz):
                q_ = p0 + g
                if q_ not in _ASET and q_ not in _GSET and q_ not in _CSET:
                    run.append(g)
                    if len(run) == 2:
                        st2 = stage.tile([NOUT, 2 * FREE], bf16, tag="st")
                        bpair[run[0]] = (st2, 0, None)
                        bpair[run[1]] = (st2, 1, run[0])
                        run = []
                else:
                    run = []
            def emit_patch(eng_, g_, p_):
                # diagonal of local row l sits at free offset RCH*p + k*257
                eng_.tensor_scalar_mul(
                    o_sb[
                        :,
                        g_ * FREE + RCH * p_ : g_ * FREE
                        + RCH * p_
                        + (RCH - 1) * (N + 1)
                        + 1 : N + 1,
                    ],
                    dv_bf[:, RCH * p_ : RCH * (p_ + 1)],
                    1.0,
                )

            for g in range(gsz):
                p = p0 + g
                defer_patch = False
                o_view = o_sb[:, g * FREE : (g + 1) * FREE].rearrange(
                    "p (k j) -> p k j", k=RCH
                )
                if p in _GSET:
                    bc = bcp.tile([NOUT, FREE], bf16, tag="bc")
                    nc.gpsimd.partition_broadcast(
                        bc[:], af_sb[:, FREE * p : FREE * (p + 1)]
                    )
                    nc.vector.tensor_mul(
                        o_view, bc[:].rearrange("p (k j) -> p k j", k=RCH), u16_rep
                    )
                    peng = nc.vector
                else:
                    ps_b = psum.tile([NOUT, FREE], f32, tag="mm")
                    for q in range(FREE // 512):
                        nc.tensor.matmul(
                            ps_b[:, 512 * q : 512 * (q + 1)],
                            lhsT=sel_v,
                            rhs=pk_v[
                                :, FREE * p + 512 * q : FREE * p + 512 * (q + 1)
                            ],
                            start=True,
                            stop=True,
                        )
                    if not _uv_emitted[0]:
                        emit_uv()
                    if p in _ASET:
                        nc.vector.tensor_mul(
                            o_view,
                            ps_b[:].rearrange("p (k j) -> p k j", k=RCH),
                            u16_rep,
                        )
                    elif p in _CSET:
                        st = stage.tile([NOUT, FREE], bf16, tag="st")
                        nc.scalar.copy(st[:], ps_b[:])
                        nc.gpsimd.tensor_mul(
                            o_view,
                            st[:].rearrange("p (k j) -> p k j", k=RCH),
                            u16_rep,
                        )
                    elif g in bpair:
                        st2, half, first_g = bpair[g]
                        nc.scalar.copy(
                            st2[:, half * FREE : (half + 1) * FREE], ps_b[:]
                        )
                        if half == 1:
                            # both halves staged: one fused 2-chunk multiply,
                            # then the deferred patch of the pair's first
                            # chunk (it must come after the fused multiply)
                            nc.vector.tensor_mul(
                                o_sb[
                                    :, first_g * FREE : (first_g + 2) * FREE
                                ].rearrange("p (k j) -> p k j", k=2 * RCH),
                                st2[:].rearrange("p (k j) -> p k j", k=2 * RCH),
                                u16_rep2,
                            )
                            emit_patch(patch_eng, first_g, p0 + first_g)
                        else:
                            defer_patch = True
                    else:
                        st = stage.tile([NOUT, FREE], bf16, tag="st")
                        nc.scalar.copy(st[:], ps_b[:])
                        nc.vector.tensor_mul(
                            o_view,
                            st[:].rearrange("p (k j) -> p k j", k=RCH),
                            u16_rep,
                        )
                    peng = patch_eng
                if not defer_patch:
                    emit_patch(peng, g, p)
            eng = nc.sync if oi % 2 == 0 else nc.gpsimd
            eng.dma_start(
                out=out[:, FREE * p0 : FREE * (p0 + gsz)], in_=o_sb[:]
            )

    nc.compile()
    _cached[key] = nc
    return nc


def _split_fp8_terms(x):
    """Split fp32 array (values in [0,1)) into 4 e4m3 terms with scales
    (1, 2^4, 2^8, 2^8) whose descaled f32 sum reconstructs x to ~4e-6."""
    import ml_dtypes

    f8 = ml_dtypes.float8_e4m3
    t0 = x.astype(f8)
    r = x - t0.astype(np.float32)
    t1 = (r * 16.0).astype(f8)
    r = r - t1.astype(np.float32) / 16.0
    t2 = (r * 256.0).astype(f8)
    r = r - t2.astype(np.float32) / 256.0
    t3 = (r * 256.0).astype(f8)
    return t0, t1, t2, t3


def _in_maps(adj, node, Wi, Wj):
    import ml_dtypes

    f8 = ml_dtypes.float8_e4m3
    sel = np.empty((KP, NOUT), f8)
    for t, s in enumerate([1.0, 2.0**-4, 2.0**-8, 2.0**-8]):
        sel[t, :] = s
    bf = ml_dtypes.bfloat16
    ckf = np.empty((NIN, N + 2 * NOUT + RPC), bf)
    ckf[:, N : N + NOUT] = Wj.T
    ckf[:, N + NOUT : N + 2 * NOUT] = Wi.T
    bf = ml_dtypes.bfloat16
    maps = []
    for c in range(NCORES):
        b, h = divmod(c, 2)
        r0 = RPC * h
        a = adj[b, 0, r0 : r0 + RPC, :]
        if h:
            ar = np.roll(a, -r0, axis=1)
            noder = np.roll(node[b], -r0, axis=1)
        else:
            ar = a
            noder = node[b]
        t0, t1, t2, t3 = _split_fp8_terms(ar.reshape(1, RPC * N))
        pk8 = np.empty((KP, CH * FREE + NOUT), f8)
        pk8[0, 0 : CH * FREE] = t0[0]
        pk8[1, 0 : CH * FREE] = t1[0]
        pk8[2, 0 : CH * FREE] = t2[0]
        pk8[3, 0 : CH * FREE] = t3[0]
        pk8[:, CH * FREE :] = sel
        m_ckf = ckf.copy()
        m_ckf[:, 0:N] = noder
        adiag = a[np.arange(RPC), r0 + np.arange(RPC)]
        m_ckf[:, N + 2 * NOUT :] = noder[:, 0:RPC] * adiag[None, :]
        af = ar.reshape(1, RPC * N).astype(bf)
        maps.append({"pk8": pk8, "ckf": m_ckf, "af": af})
    return maps


def kernel(**inputs):
    global last_results
    adj = np.asarray(inputs["adj"], dtype=np.float32)
    node = np.asarray(inputs["node"], dtype=np.float32)
    Wi = np.asarray(inputs["Wi"], dtype=np.float32)
    Wj = np.asarray(inputs["Wj"], dtype=np.float32)

    from concourse.bass_utils import run_bass_kernel_spmd

    nc = _build_nc()
    res = run_bass_kernel_spmd(nc, _in_maps(adj, node, Wi, Wj), list(range(NCORES)))
    last_results = res

    out = np.empty((B, NOUT, N, N), np.float32)
    for c in range(NCORES):
        b, h = divmod(c, 2)
        co = res.results[c]["out"].astype(np.float32).reshape(NOUT, RPC, N)
        if h:
            co = np.roll(co, RPC * h, axis=2)
        out[b, :, RPC * h : RPC * (h + 1), :] = co
    return out


# revision 34
# speedup vs baseline: 1.0531x; 1.0197x over previous
"""NodeConv kernel for 8 Trainium2 NeuronCores.

Reference computes, for adj [B,1,N,N], node [B,nin,N], Wi/Wj [nout,nin]:
    x  = node[:, :, None, :] * adj          # [B,nin,N,N]
    yi = einsum('oc,bcij->boij', Wi, x)
    yj = einsum('oc,bcij->boij', Wj, x)
    out = I * yi + (1-I) * yj

Because adj[b,i,j] does not depend on the contraction channel c, the
contraction factors out:
    off-diag: out[b,o,i,j] = adj[b,i,j] * (Wj @ node[b])[o,j]
    diag:     out[b,o,j,j] = adj[b,j,j] * (Wi @ node[b])[o,j]

Sharding: core c handles batch b=c//2, row half h=c%2 (128 rows). Odd
halves get their columns rolled by -128 on the host so the diagonal of
local row l sits at local column l on every core -> one SPMD program;
the host rolls the output back while gathering.

Design (measured down from the 72.8us f32 baseline to ~54us; tolerance
is 2e-2 and this path measures ~7.4e-3 max rel err):
  - OUTPUT IS BF16 (host upconverts to f32 while gathering): halves the
    HBM store traffic 16 MiB -> 8 MiB per core.  DMA floor ~21us.
  - adj is split host-side into 4 scaled e4m3 terms (reconstruction
    error ~4e-6) stored on psum partitions 0-3, chunk-major along the
    free dim, with a constant [4, NOUT] stationary holding the descale
    factors (1, 2^-4, 2^-8, 2^-8).  Each 512-col fp8 matmul broadcasts
    adj to all 128 output partitions in one pass (PE streams 1 col/cycle
    at the 1.2 GHz mid p-state regardless of dtype, so K and dtype are
    chosen purely to minimize the input load: 128 KB).
  - weights/node ship as bf16 so the u = Wj@node and dv = Wi@nodeD
    matmuls are single-pass (fp32 PE matmuls run as 2 half-speed
    passes).  nodeD is the diagonal node columns prescaled by adj's
    diagonal on the host, so dv IS the diagonal patch value - no
    diag-broadcast matmul needed.
  - per chunk (4 rows x 256 = 1024 cols, [128,1024] f32 psum tiles,
    4-buffer rotation so the PE->consumer round-trip never idles the
    pipeline): A-chunks are multiplied straight from PSUM on DVE
    (f32 x bf16 -> bf16, 1 elem/lane/cycle); B-chunks are copied
    PSUM f32 -> bf16 SBUF by ScalarE, then DVE multiplies all-bf16 in
    2x_1P mode (2 elem/lane/cycle).  The A/B mix balances ScalarE
    (~26us) against DVE (~27us) - the body floor.
  - diagonal patches (strided [128, RCH] writes of dv) run on the
    otherwise-idle GpSimd engine.
  - stores are issued from the Sync and GpSimd queues, group sizes
    tapered (small first groups start the DMA flow early, small last
    groups shorten the final drain); NODECONV_GORDER permutes group
    processing order.
  - GSET chunks (off by default) use GpSimd partition_broadcast of a
    host-rounded bf16 adj instead of the PE path; measured slower
    (~4.1us per chunk at 0.41 efficiency) but kept as a knob.
"""

import os

import numpy as np

NCORES = 8
B, N, NIN, NOUT = 4, 256, 128, 128
RPC = 128          # rows per core
RCH = int(os.environ.get("NODECONV_RCH", "4"))   # rows per chunk
CH = RPC // RCH    # chunks per core
FREE = RCH * N     # free elems per chunk
PSUM_BUFS = int(os.environ.get("NODECONV_PSUM_BUFS", "4" if RCH <= 4 else "2"))

# G-chunks: GpSimd partition_broadcast of host-rounded bf16 adj + DVE 2x_1P
# multiply — no PE/PSUM/ScalarE involvement.  Placed at the edges so the
# first stores fire early and the tail chunks are compute-ready early.
_GSET = {
    int(x)
    for x in os.environ.get("NODECONV_GSET", "").split(",")
    if x != ""
}
# C-chunks: ScalarE bf16 staging copy + GpSimd (standard-library) multiply —
# relieves the saturated DVE using GpSimd idle time.  Patches for these run
# in-order on GpSimd right after the multiply (no cross-engine semaphore).
_CSET = {
    int(x)
    for x in os.environ.get(
        "NODECONV_CSET", ""
    ).split(",")
    if x != ""
}
# A-chunks: multiplied directly from PSUM on DVE (1x); remaining (B) chunks
# go through a ScalarE bf16 staging copy + DVE 2x_1P multiply
_ASET = {
    int(x)
    for x in os.environ.get(
        "NODECONV_ASET",
        "0,4,8,12,16,20,24,27,30,31" if RCH == 4 else "0,5,10,15",
    ).split(",")
    if x != ""
}
_G = [
    int(x)
    for x in os.environ.get(
        "NODECONV_GROUPS",
        "1,1,2,2,4,4,4,4,4,2,2,2" if RCH == 4 else "2,2,2,2,2,2,2,1,1",
    ).split(",")
]
assert sum(_G) == CH
# processing order of the groups: DRAM-tail groups run early so the final
# store is not serialized behind the final DRAM addresses; the
# last-processed group is small for a short drain
_GORDER = [
    int(x)
    for x in os.environ.get(
        "NODECONV_GORDER",
        "0,1,10,11,2,3,4,5,6,7,8,9" if RCH == 4 else ",".join(map(str, range(9))),
    ).split(",")
]
assert sorted(_GORDER) == list(range(len(_G)))
STAGE_BUFS = int(os.environ.get("NODECONV_STAGE_BUFS", "6"))
OUT_BUFS = int(os.environ.get("NODECONV_OUT_BUFS", "6"))
PATCH_ENG = os.environ.get("NODECONV_PATCH", "gp")  # gp | scalar | vector

KP = 4             # fp8 term partitions (t0..t3)

_cached = {}

last_results = None  # BassKernelResults of the most recent kernel() call


def _build_nc():
    key = (
        RCH,
        PSUM_BUFS,
        tuple(sorted(_ASET)),
        tuple(sorted(_CSET)),
        tuple(sorted(_GSET)),
        tuple(_G),
        tuple(_GORDER),
        STAGE_BUFS,
        OUT_BUFS,
        PATCH_ENG,
    )
    if key in _cached:
        return _cached[key]

    from contextlib import ExitStack

    import concourse.tile as tile
    from concourse import bacc, mybir

    f32 = mybir.dt.float32
    bf16 = mybir.dt.bfloat16
    fp8 = mybir.dt.float8e4

    nc = bacc.Bacc(
        "TRN2", target_bir_lowering=False, debug=False, num_devices=NCORES
    )

    # pk8: [4, CH*FREE + NOUT] fp8 — partition t holds scaled adj term t for
    # every chunk (chunk-major along the free dim), then the [4, NOUT]
    # stationary selector carrying the descale factors (1, 2^-4, 2^-8,
    # 2^-8).  Every matmul slices this at base partition 0 with the SAME
    # stationary, so no per-chunk selector blocks are needed.
    pk8 = nc.dram_tensor(
        "pk8", [KP, CH * FREE + NOUT], fp8, kind="ExternalInput"
    ).ap()
    # ckf: [128, 640] bf16 — node_r | WiT | WjT | nodeD, where nodeD is the
    # diagonal node columns prescaled by adj's diagonal on the host, so
    # Wi @ nodeD directly yields the diagonal patch values dv.  bf16 keeps
    # the u/dv matmuls single-pass (fp32 PE matmuls run as 2 half-speed
    # passes) and halves the critical first input load.
    ckf = nc.dram_tensor(
        "ckf", [NIN, N + 2 * NOUT + RPC], bf16, kind="ExternalInput"
    ).ap()
    # af: chunk p's eight adj rows flattened on partition 0 (bf16) — the
    # source for GpSimd partition_broadcast on G-chunks
    af = nc.dram_tensor("af", [1, RPC * N], bf16, kind="ExternalInput").ap()
    out = nc.dram_tensor("out", [NOUT, RPC * N], bf16, kind="ExternalOutput").ap()

    with tile.TileContext(nc) as tc, ExitStack() as ctx:
        const = ctx.enter_context(tc.tile_pool(name="const", bufs=1))
        psum = ctx.enter_context(tc.tile_pool(name="psum", bufs=PSUM_BUFS, space="PSUM"))
        outp = ctx.enter_context(tc.tile_pool(name="outp", bufs=OUT_BUFS))
        stage = ctx.enter_context(tc.tile_pool(name="stage", bufs=STAGE_BUFS))
        bcp = ctx.enter_context(tc.tile_pool(name="bcp", bufs=len(_GSET) or 1))

        # Input loads are split across the sync and gpsimd queues: the
        # 4-partition pk8 tensor transfers slowly per descriptor, so each
        # queue carries half, and ckf's u-critical piece (node|WjT) plus the
        # matmul selector go out first on sync.
        ckf_sb = const.tile([NIN, N + 2 * NOUT + RPC], bf16)
        pk8_sb = const.tile([KP, CH * FREE + NOUT], fp8)
        # Pieces land in processing order (GORDER runs chunks 0,1,28..31,
        # 2,3,... first): the tiny selector + chunks 0-1 go first on sync so
        # the first matmul fires ~3us earlier; ckf halves lead each queue's
        # bulk so the u/dv chain and the early-processed tail chunks are
        # never the gating load.
        def dma(eng, lo, hi):
            eng.dma_start(out=pk8_sb[:, lo:hi], in_=pk8[:, lo:hi])

        dma(nc.sync, CH * FREE, CH * FREE + NOUT)          # selector
        dma(nc.sync, 0, 2 * FREE)                          # chunks 0-1
        nc.sync.dma_start(out=ckf_sb[:, 0 : N + NOUT], in_=ckf[:, 0 : N + NOUT])
        nc.gpsimd.dma_start(
            out=ckf_sb[:, N + NOUT :], in_=ckf[:, N + NOUT :]
        )
        dma(nc.gpsimd, 28 * FREE, 32 * FREE)               # chunks 28-31
        dma(nc.gpsimd, 2 * FREE, 16 * FREE)                # chunks 2-15
        dma(nc.sync, 16 * FREE, 28 * FREE)                 # chunks 16-27
        if _GSET:
            af_sb = const.tile([1, RPC * N], bf16)
            nc.gpsimd.dma_start(out=af_sb[:], in_=af)

        node_sb = ckf_sb[:, 0:N]
        wjt_sb = ckf_sb[:, N : N + NOUT]
        wit_sb = ckf_sb[:, N + NOUT : N + 2 * NOUT]
        noded_sb = ckf_sb[:, N + 2 * NOUT : N + 2 * NOUT + RPC]
        sel_v = pk8_sb[:, CH * FREE : CH * FREE + NOUT]
        pk_v = pk8_sb[:, 0 : CH * FREE]

        # u = Wj @ node_r -> [nout, N], and (later) dv = Wi @ nodeD.  Both
        # live in one psum tile so the chunk psum tiles keep alternating
        # between the pool's two buffers.
        u_bf = const.tile([NOUT, N], bf16)
        dv_bf = const.tile([NOUT, RPC], bf16)
        _uv_emitted = [False]

        def emit_uv():
            # u = Wj @ node_r and dv = Wi @ nodeD, emitted right after the
            # first chunk's matmuls so the pk8-gated broadcast work heads
            # the PE queue; the dv copy precedes every patch in program
            # order (patches read dv_bf)
            ps_uv = psum.tile([NOUT, N + RPC], f32, tag="mm")
            nc.tensor.matmul(
                ps_uv[:, 0:N], lhsT=wjt_sb, rhs=node_sb, start=True, stop=True
            )
            nc.scalar.copy(u_bf[:], ps_uv[:, 0:N])
            nc.tensor.matmul(
                ps_uv[:, N : N + RPC],
                lhsT=wit_sb,
                rhs=noded_sb,
                start=True,
                stop=True,
            )
            nc.scalar.copy(dv_bf[:], ps_uv[:, N : N + RPC])
            _uv_emitted[0] = True

        # u replicated RCH (and 2*RCH for fused B-pairs) times along the
        # free dim via stride-0 views
        u16_rep = u_bf[:].unsqueeze(1).broadcast_to([NOUT, RCH, N])
        u16_rep2 = u_bf[:].unsqueeze(1).broadcast_to([NOUT, 2 * RCH, N])

        patch_eng = {
            "gp": nc.gpsimd,
            "scalar": nc.scalar,
            "vector": nc.vector,
        }[PATCH_ENG]

        gstart = [0]
        for gsz in _G:
            gstart.append(gstart[-1] + gsz)
        for oi, gi in enumerate(_GORDER):
            gsz = _G[gi]
            o_sb = outp.tile([NOUT, gsz * FREE], bf16, tag="osb")
            p0 = gstart[gi]
            # adjacent B-chunks within a group share one stage tile and a
            # single fused DVE multiply over both (fewer DVE instructions)
            bpair = {}
            run = []
            for g in range(gsz):
                q_ = p0 + g
                if q_ not in _ASET and q_ not in _GSET and q_ not in _CSET:
                    run.append(g)
                    if len(run) == 2:
                        st2 = stage.tile([NOUT, 2 * FREE], bf16, tag="st")
                        bpair[run[0]] = (st2, 0, None)
                        bpair[run[1]] = (st2, 1, run[0])
                        run = []
                else:
                    run = []
            def emit_patch(eng_, g_, p_):
                # diagonal of local row l sits at free offset RCH*p + k*257
                eng_.tensor_scalar_mul(
                    o_sb[
                        :,
                        g_ * FREE + RCH * p_ : g_ * FREE
                        + RCH * p_
                        + (RCH - 1) * (N + 1)
                        + 1 : N + 1,
                    ],
                    dv_bf[:, RCH * p_ : RCH * (p_ + 1)],
                    1.0,
                )

            for g in range(gsz):
                p = p0 + g
                defer_patch = False
                o_view = o_sb[:, g * FREE : (g + 1) * FREE].rearrange(
                    "p (k j) -> p k j", k=RCH
                )
                if p in _GSET:
                    bc = bcp.tile([NOUT, FREE], bf16, tag="bc")
                    nc.gpsimd.partition_broadcast(
                        bc[:], af_sb[:, FREE * p : FREE * (p + 1)]
                    )
                    nc.vector.tensor_mul(
                        o_view, bc[:].rearrange("p (k j) -> p k j", k=RCH), u16_rep
                    )
                    peng = nc.vector
                else:
                    ps_b = psum.tile([NOUT, FREE], f32, tag="mm")
                    for q in range(FREE // 512):
                        nc.tensor.matmul(
                            ps_b[:, 512 * q : 512 * (q + 1)],
                            lhsT=sel_v,
                            rhs=pk_v[
                                :, FREE * p + 512 * q : FREE * p + 512 * (q + 1)
                            ],
                            start=True,
                            stop=True,
                        )
                    if not _uv_emitted[0]:
                        emit_uv()
                    if p in _ASET:
                        nc.vector.tensor_mul(
                            o_view,
                            ps_b[:].rearrange("p (k j) -> p k j", k=RCH),
                            u16_rep,
                        )
                    elif p in _CSET:
                        st = stage.tile([NOUT, FREE], bf16, tag="st")
                        nc.scalar.copy(st[:], ps_b[:])
                        nc.gpsimd.tensor_mul(
                            o_view,
                            st[:].rearrange("p (k j) -> p k j", k=RCH),
                            u16_rep,
                        )
                    elif g in bpair:
                        st2, half, first_g = bpair[g]
                        nc.scalar.copy(
                            st2[:, half * FREE : (half + 1) * FREE], ps_b[:]
                        )
                        if half == 1:
                            # both halves staged: one fused 2-chunk multiply,
                            # then the deferred patch of the pair's first
                            # chunk (it must come after the fused multiply)
                            nc.vector.tensor_mul(
                                o_sb[
                                    :, first_g * FREE : (first_g + 2) * FREE
                                ].rearrange("p (k j) -> p k j", k=2 * RCH),
                                st2[:].rearrange("p (k j) -> p k j", k=2 * RCH),
                                u16_rep2,
                            )
                            emit_patch(patch_eng, first_g, p0 + first_g)
                        else:
                            defer_patch = True
                    else:
                        st = stage.tile([NOUT, FREE], bf16, tag="st")
                        nc.scalar.copy(st[:], ps_b[:])
                        nc.vector.tensor_mul(
                            o_view,
                            st[:].rearrange("p (k j) -> p k j", k=RCH),
                            u16_rep,
                        )
                    peng = patch_eng
                if not defer_patch:
                    emit_patch(peng, g, p)
            eng = nc.sync if oi % 2 == 0 else nc.gpsimd
            eng.dma_start(
                out=out[:, FREE * p0 : FREE * (p0 + gsz)], in_=o_sb[:]
            )

    nc.compile()
    _cached[key] = nc
    return nc


def _split_fp8_terms(x):
    """Split fp32 array (values in [0,1)) into 4 e4m3 terms with scales
    (1, 2^4, 2^8, 2^8) whose descaled f32 sum reconstructs x to ~4e-6."""
    import ml_dtypes

    f8 = ml_dtypes.float8_e4m3
    t0 = x.astype(f8)
    r = x - t0.astype(np.float32)
    t1 = (r * 16.0).astype(f8)
    r = r - t1.astype(np.float32) / 16.0
    t2 = (r * 256.0).astype(f8)
    r = r - t2.astype(np.float32) / 256.0
    t3 = (r * 256.0).astype(f8)
    return t0, t1, t2, t3


def _in_maps(adj, node, Wi, Wj):
    import ml_dtypes

    f8 = ml_dtypes.float8_e4m3
    sel = np.empty((KP, NOUT), f8)
    for t, s in enumerate([1.0, 2.0**-4, 2.0**-8, 2.0**-8]):
        sel[t, :] = s
    bf = ml_dtypes.bfloat16
    ckf = np.empty((NIN, N + 2 * NOUT + RPC), bf)
    ckf[:, N : N + NOUT] = Wj.T
    ckf[:, N + NOUT : N + 2 * NOUT] = Wi.T
    bf = ml_dtypes.bfloat16
    maps = []
    for c in range(NCORES):
        b, h = divmod(c, 2)
        r0 = RPC * h
        a = adj[b, 0, r0 : r0 + RPC, :]
        if h:
            ar = np.roll(a, -r0, axis=1)
            noder = np.roll(node[b], -r0, axis=1)
        else:
            ar = a
            noder = node[b]
        t0, t1, t2, t3 = _split_fp8_terms(ar.reshape(1, RPC * N))
        pk8 = np.empty((KP, CH * FREE + NOUT), f8)
        pk8[0, 0 : CH * FREE] = t0[0]
        pk8[1, 0 : CH * FREE] = t1[0]
        pk8[2, 0 : CH * FREE] = t2[0]
        pk8[3, 0 : CH * FREE] = t3[0]
        pk8[:, CH * FREE :] = sel
        m_ckf = ckf.copy()
        m_ckf[:, 0:N] = noder
        adiag = a[np.arange(RPC), r0 + np.arange(RPC)]
        m_ckf[:, N + 2 * NOUT :] = noder[:, 0:RPC] * adiag[None, :]
        af = ar.reshape(1, RPC * N).astype(bf)
        maps.append({"pk8": pk8, "ckf": m_ckf, "af": af})
    return maps


def kernel(**inputs):
    global last_results
    adj = np.asarray(inputs["adj"], dtype=np.float32)
    node = np.asarray(inputs["node"], dtype=np.float32)
    Wi = np.asarray(inputs["Wi"], dtype=np.float32)
    Wj = np.asarray(inputs["Wj"], dtype=np.float32)

    from concourse.bass_utils import run_bass_kernel_spmd

    nc = _build_nc()
    res = run_bass_kernel_spmd(nc, _in_maps(adj, node, Wi, Wj), list(range(NCORES)))
    last_results = res

    out = np.empty((B, NOUT, N, N), np.float32)
    for c in range(NCORES):
        b, h = divmod(c, 2)
        co = res.results[c]["out"].astype(np.float32).reshape(NOUT, RPC, N)
        if h:
            co = np.roll(co, RPC * h, axis=2)
        out[b, :, RPC * h : RPC * (h + 1), :] = co
    return out


# revision 35
# speedup vs baseline: 1.0867x; 1.0319x over previous
"""NodeConv kernel for 8 Trainium2 NeuronCores.

Reference computes, for adj [B,1,N,N], node [B,nin,N], Wi/Wj [nout,nin]:
    x  = node[:, :, None, :] * adj          # [B,nin,N,N]
    yi = einsum('oc,bcij->boij', Wi, x)
    yj = einsum('oc,bcij->boij', Wj, x)
    out = I * yi + (1-I) * yj

Because adj[b,i,j] does not depend on the contraction channel c, the
contraction factors out:
    off-diag: out[b,o,i,j] = adj[b,i,j] * (Wj @ node[b])[o,j]
    diag:     out[b,o,j,j] = adj[b,j,j] * (Wi @ node[b])[o,j]

Sharding: core c handles batch b=c//2, row half h=c%2 (128 rows). Odd
halves get their columns rolled by -128 on the host so the diagonal of
local row l sits at local column l on every core -> one SPMD program;
the host rolls the output back while gathering.

Design (measured down from the 72.8us f32 baseline to ~54us; tolerance
is 2e-2 and this path measures ~7.4e-3 max rel err):
  - OUTPUT IS BF16 (host upconverts to f32 while gathering): halves the
    HBM store traffic 16 MiB -> 8 MiB per core.  DMA floor ~21us.
  - adj is split host-side into 4 scaled e4m3 terms (reconstruction
    error ~4e-6) stored on psum partitions 0-3, chunk-major along the
    free dim, with a constant [4, NOUT] stationary holding the descale
    factors (1, 2^-4, 2^-8, 2^-8).  Each 512-col fp8 matmul broadcasts
    adj to all 128 output partitions in one pass (PE streams 1 col/cycle
    at the 1.2 GHz mid p-state regardless of dtype, so K and dtype are
    chosen purely to minimize the input load: 128 KB).
  - weights/node ship as bf16 so the u = Wj@node and dv = Wi@nodeD
    matmuls are single-pass (fp32 PE matmuls run as 2 half-speed
    passes).  nodeD is the diagonal node columns prescaled by adj's
    diagonal on the host, so dv IS the diagonal patch value - no
    diag-broadcast matmul needed.
  - per chunk (4 rows x 256 = 1024 cols, [128,1024] f32 psum tiles,
    4-buffer rotation so the PE->consumer round-trip never idles the
    pipeline): A-chunks are multiplied straight from PSUM on DVE
    (f32 x bf16 -> bf16, 1 elem/lane/cycle); B-chunks are copied
    PSUM f32 -> bf16 SBUF by ScalarE, then DVE multiplies all-bf16 in
    2x_1P mode (2 elem/lane/cycle).  The A/B mix balances ScalarE
    (~26us) against DVE (~27us) - the body floor.
  - diagonal patches (strided [128, RCH] writes of dv) run on the
    otherwise-idle GpSimd engine.
  - stores are issued from the Sync and GpSimd queues, group sizes
    tapered (small first groups start the DMA flow early, small last
    groups shorten the final drain); NODECONV_GORDER permutes group
    processing order.
  - GSET chunks (off by default) use GpSimd partition_broadcast of a
    host-rounded bf16 adj instead of the PE path; measured slower
    (~4.1us per chunk at 0.41 efficiency) but kept as a knob.
"""

import os

import numpy as np

NCORES = 8
B, N, NIN, NOUT = 4, 256, 128, 128
RPC = 128          # rows per core
RCH = int(os.environ.get("NODECONV_RCH", "4"))   # rows per chunk
CH = RPC // RCH    # chunks per core
FREE = RCH * N     # free elems per chunk
PSUM_BUFS = int(os.environ.get("NODECONV_PSUM_BUFS", "4" if RCH <= 4 else "2"))

# G-chunks: GpSimd partition_broadcast of host-rounded bf16 adj + DVE 2x_1P
# multiply — no PE/PSUM/ScalarE involvement.  Placed at the edges so the
# first stores fire early and the tail chunks are compute-ready early.
_GSET = {
    int(x)
    for x in os.environ.get("NODECONV_GSET", "").split(",")
    if x != ""
}
# C-chunks: ScalarE bf16 staging copy + GpSimd (standard-library) multiply —
# relieves the saturated DVE using GpSimd idle time.  Patches for these run
# in-order on GpSimd right after the multiply (no cross-engine semaphore).
_CSET = {
    int(x)
    for x in os.environ.get(
        "NODECONV_CSET", ""
    ).split(",")
    if x != ""
}
# A-chunks: multiplied directly from PSUM on DVE (1x); remaining (B) chunks
# go through a ScalarE bf16 staging copy + DVE 2x_1P multiply
_ASET = {
    int(x)
    for x in os.environ.get(
        "NODECONV_ASET",
        "0,4,8,12,16,20,24,27,30,31" if RCH == 4 else "0,5,10,15",
    ).split(",")
    if x != ""
}
_G = [
    int(x)
    for x in os.environ.get(
        "NODECONV_GROUPS",
        "1,1,2,2,4,4,4,4,4,2,2,2" if RCH == 4 else "2,2,2,2,2,2,2,1,1",
    ).split(",")
]
assert sum(_G) == CH
# processing order of the groups: the LAST-processed groups are the
# single-chunk ones (and the final chunk is A-type, the shortest
# mms->mul->patch->store chain), so almost no work remains after the last
# PE matmul; both tail chunks sit in the first-loaded pk8 piece
_GORDER = [
    int(x)
    for x in os.environ.get(
        "NODECONV_GORDER",
        "1,2,3,4,5,6,7,8,9,10,11,0" if RCH == 4 else ",".join(map(str, range(9))),
    ).split(",")
]
assert sorted(_GORDER) == list(range(len(_G)))
STAGE_BUFS = int(os.environ.get("NODECONV_STAGE_BUFS", "6"))
OUT_BUFS = int(os.environ.get("NODECONV_OUT_BUFS", "6"))
PATCH_ENG = os.environ.get("NODECONV_PATCH", "gp")  # gp | scalar | vector

KP = 4             # fp8 term partitions (t0..t3)

_cached = {}

last_results = None  # BassKernelResults of the most recent kernel() call


def _build_nc():
    key = (
        RCH,
        PSUM_BUFS,
        tuple(sorted(_ASET)),
        tuple(sorted(_CSET)),
        tuple(sorted(_GSET)),
        tuple(_G),
        tuple(_GORDER),
        STAGE_BUFS,
        OUT_BUFS,
        PATCH_ENG,
    )
    if key in _cached:
        return _cached[key]

    from contextlib import ExitStack

    import concourse.tile as tile
    from concourse import bacc, mybir

    f32 = mybir.dt.float32
    bf16 = mybir.dt.bfloat16
    fp8 = mybir.dt.float8e4

    nc = bacc.Bacc(
        "TRN2", target_bir_lowering=False, debug=False, num_devices=NCORES
    )

    # pk8: [4, CH*FREE + NOUT] fp8 — partition t holds scaled adj term t for
    # every chunk (chunk-major along the free dim), then the [4, NOUT]
    # stationary selector carrying the descale factors (1, 2^-4, 2^-8,
    # 2^-8).  Every matmul slices this at base partition 0 with the SAME
    # stationary, so no per-chunk selector blocks are needed.
    pk8 = nc.dram_tensor(
        "pk8", [KP, CH * FREE + NOUT], fp8, kind="ExternalInput"
    ).ap()
    # ckf: [128, 640] bf16 — node_r | WiT | WjT | nodeD, where nodeD is the
    # diagonal node columns prescaled by adj's diagonal on the host, so
    # Wi @ nodeD directly yields the diagonal patch values dv.  bf16 keeps
    # the u/dv matmuls single-pass (fp32 PE matmuls run as 2 half-speed
    # passes) and halves the critical first input load.
    ckf = nc.dram_tensor(
        "ckf", [NIN, N + 2 * NOUT + RPC], bf16, kind="ExternalInput"
    ).ap()
    # af: chunk p's eight adj rows flattened on partition 0 (bf16) — the
    # source for GpSimd partition_broadcast on G-chunks
    af = nc.dram_tensor("af", [1, RPC * N], bf16, kind="ExternalInput").ap()
    out = nc.dram_tensor("out", [NOUT, RPC * N], bf16, kind="ExternalOutput").ap()

    with tile.TileContext(nc) as tc, ExitStack() as ctx:
        const = ctx.enter_context(tc.tile_pool(name="const", bufs=1))
        psum = ctx.enter_context(tc.tile_pool(name="psum", bufs=PSUM_BUFS, space="PSUM"))
        outp = ctx.enter_context(tc.tile_pool(name="outp", bufs=OUT_BUFS))
        stage = ctx.enter_context(tc.tile_pool(name="stage", bufs=STAGE_BUFS))
        bcp = ctx.enter_context(tc.tile_pool(name="bcp", bufs=len(_GSET) or 1))

        # Input loads are split across the sync and gpsimd queues: the
        # 4-partition pk8 tensor transfers slowly per descriptor, so each
        # queue carries half, and ckf's u-critical piece (node|WjT) plus the
        # matmul selector go out first on sync.
        ckf_sb = const.tile([NIN, N + 2 * NOUT + RPC], bf16)
        pk8_sb = const.tile([KP, CH * FREE + NOUT], fp8)
        # Pieces land in processing order (GORDER runs chunks 0,1,28..31,
        # 2,3,... first): the tiny selector + chunks 0-1 go first on sync so
        # the first matmul fires ~3us earlier; ckf halves lead each queue's
        # bulk so the u/dv chain and the early-processed tail chunks are
        # never the gating load.
        def dma(eng, lo, hi):
            eng.dma_start(out=pk8_sb[:, lo:hi], in_=pk8[:, lo:hi])

        dma(nc.sync, CH * FREE, CH * FREE + NOUT)          # selector
        dma(nc.sync, 0, 2 * FREE)                          # chunks 0-1
        nc.sync.dma_start(out=ckf_sb[:, 0 : N + NOUT], in_=ckf[:, 0 : N + NOUT])
        nc.gpsimd.dma_start(
            out=ckf_sb[:, N + NOUT :], in_=ckf[:, N + NOUT :]
        )
        dma(nc.gpsimd, 28 * FREE, 32 * FREE)               # chunks 28-31
        dma(nc.gpsimd, 2 * FREE, 16 * FREE)                # chunks 2-15
        dma(nc.sync, 16 * FREE, 28 * FREE)                 # chunks 16-27
        if _GSET:
            af_sb = const.tile([1, RPC * N], bf16)
            nc.gpsimd.dma_start(out=af_sb[:], in_=af)

        node_sb = ckf_sb[:, 0:N]
        wjt_sb = ckf_sb[:, N : N + NOUT]
        wit_sb = ckf_sb[:, N + NOUT : N + 2 * NOUT]
        noded_sb = ckf_sb[:, N + 2 * NOUT : N + 2 * NOUT + RPC]
        sel_v = pk8_sb[:, CH * FREE : CH * FREE + NOUT]
        pk_v = pk8_sb[:, 0 : CH * FREE]

        # u = Wj @ node_r -> [nout, N], and (later) dv = Wi @ nodeD.  Both
        # live in one psum tile so the chunk psum tiles keep alternating
        # between the pool's two buffers.
        u_bf = const.tile([NOUT, N], bf16)
        dv_bf = const.tile([NOUT, RPC], bf16)
        _uv_emitted = [False]

        def emit_uv():
            # u = Wj @ node_r and dv = Wi @ nodeD, emitted right after the
            # first chunk's matmuls so the pk8-gated broadcast work heads
            # the PE queue; the dv copy precedes every patch in program
            # order (patches read dv_bf)
            ps_uv = psum.tile([NOUT, N + RPC], f32, tag="mm")
            nc.tensor.matmul(
                ps_uv[:, 0:N], lhsT=wjt_sb, rhs=node_sb, start=True, stop=True
            )
            nc.scalar.copy(u_bf[:], ps_uv[:, 0:N])
            nc.tensor.matmul(
                ps_uv[:, N : N + RPC],
                lhsT=wit_sb,
                rhs=noded_sb,
                start=True,
                stop=True,
            )
            nc.scalar.copy(dv_bf[:], ps_uv[:, N : N + RPC])
            _uv_emitted[0] = True

        # u replicated RCH (and 2*RCH for fused B-pairs) times along the
        # free dim via stride-0 views
        u16_rep = u_bf[:].unsqueeze(1).broadcast_to([NOUT, RCH, N])
        u16_rep2 = u_bf[:].unsqueeze(1).broadcast_to([NOUT, 2 * RCH, N])

        patch_eng = {
            "gp": nc.gpsimd,
            "scalar": nc.scalar,
            "vector": nc.vector,
        }[PATCH_ENG]

        gstart = [0]
        for gsz in _G:
            gstart.append(gstart[-1] + gsz)
        for oi, gi in enumerate(_GORDER):
            gsz = _G[gi]
            o_sb = outp.tile([NOUT, gsz * FREE], bf16, tag="osb")
            p0 = gstart[gi]
            # adjacent B-chunks within a group share one stage tile and a
            # single fused DVE multiply over both (fewer DVE instructions)
            bpair = {}
            run = []
            for g in range(gsz):
                q_ = p0 + g
                if q_ not in _ASET and q_ not in _GSET and q_ not in _CSET:
                    run.append(g)
                    if len(run) == 2:
                        st2 = stage.tile([NOUT, 2 * FREE], bf16, tag="st")
                        bpair[run[0]] = (st2, 0, None)
                        bpair[run[1]] = (st2, 1, run[0])
                        run = []
                else:
                    run = []
            def emit_patch(eng_, g_, p_):
                # diagonal of local row l sits at free offset RCH*p + k*257
                eng_.tensor_scalar_mul(
                    o_sb[
                        :,
                        g_ * FREE + RCH * p_ : g_ * FREE
                        + RCH * p_
                        + (RCH - 1) * (N + 1)
                        + 1 : N + 1,
                    ],
                    dv_bf[:, RCH * p_ : RCH * (p_ + 1)],
                    1.0,
                )

            for g in range(gsz):
                p = p0 + g
                defer_patch = False
                o_view = o_sb[:, g * FREE : (g + 1) * FREE].rearrange(
                    "p (k j) -> p k j", k=RCH
                )
                if p in _GSET:
                    bc = bcp.tile([NOUT, FREE], bf16, tag="bc")
                    nc.gpsimd.partition_broadcast(
                        bc[:], af_sb[:, FREE * p : FREE * (p + 1)]
                    )
                    nc.vector.tensor_mul(
                        o_view, bc[:].rearrange("p (k j) -> p k j", k=RCH), u16_rep
                    )
                    peng = nc.vector
                else:
                    ps_b = psum.tile([NOUT, FREE], f32, tag="mm")
                    for q in range(FREE // 512):
                        nc.tensor.matmul(
                            ps_b[:, 512 * q : 512 * (q + 1)],
                            lhsT=sel_v,
                            rhs=pk_v[
                                :, FREE * p + 512 * q : FREE * p + 512 * (q + 1)
                            ],
                            start=True,
                            stop=True,
                        )
                    if not _uv_emitted[0]:
                        emit_uv()
                    if p in _ASET:
                        nc.vector.tensor_mul(
                            o_view,
                            ps_b[:].rearrange("p (k j) -> p k j", k=RCH),
                            u16_rep,
                        )
                    elif p in _CSET:
                        st = stage.tile([NOUT, FREE], bf16, tag="st")
                        nc.scalar.copy(st[:], ps_b[:])
                        nc.gpsimd.tensor_mul(
                            o_view,
                            st[:].rearrange("p (k j) -> p k j", k=RCH),
                            u16_rep,
                        )
                    elif g in bpair:
                        st2, half, first_g = bpair[g]
                        nc.scalar.copy(
                            st2[:, half * FREE : (half + 1) * FREE], ps_b[:]
                        )
                        if half == 1:
                            # both halves staged: one fused 2-chunk multiply,
                            # then the deferred patch of the pair's first
                            # chunk (it must come after the fused multiply)
                            nc.vector.tensor_mul(
                                o_sb[
                                    :, first_g * FREE : (first_g + 2) * FREE
                                ].rearrange("p (k j) -> p k j", k=2 * RCH),
                                st2[:].rearrange("p (k j) -> p k j", k=2 * RCH),
                                u16_rep2,
                            )
                            emit_patch(patch_eng, first_g, p0 + first_g)
                        else:
                            defer_patch = True
                    else:
                        st = stage.tile([NOUT, FREE], bf16, tag="st")
                        nc.scalar.copy(st[:], ps_b[:])
                        nc.vector.tensor_mul(
                            o_view,
                            st[:].rearrange("p (k j) -> p k j", k=RCH),
                            u16_rep,
                        )
                    peng = patch_eng
                if not defer_patch:
                    emit_patch(peng, g, p)
            eng = nc.sync if oi % 2 == 0 else nc.gpsimd
            eng.dma_start(
                out=out[:, FREE * p0 : FREE * (p0 + gsz)], in_=o_sb[:]
            )

    nc.compile()
    _cached[key] = nc
    return nc


def _split_fp8_terms(x):
    """Split fp32 array (values in [0,1)) into 4 e4m3 terms with scales
    (1, 2^4, 2^8, 2^8) whose descaled f32 sum reconstructs x to ~4e-6."""
    import ml_dtypes

    f8 = ml_dtypes.float8_e4m3
    t0 = x.astype(f8)
    r = x - t0.astype(np.float32)
    t1 = (r * 16.0).astype(f8)
    r = r - t1.astype(np.float32) / 16.0
    t2 = (r * 256.0).astype(f8)
    r = r - t2.astype(np.float32) / 256.0
    t3 = (r * 256.0).astype(f8)
    return t0, t1, t2, t3


def _in_maps(adj, node, Wi, Wj):
    import ml_dtypes

    f8 = ml_dtypes.float8_e4m3
    sel = np.empty((KP, NOUT), f8)
    for t, s in enumerate([1.0, 2.0**-4, 2.0**-8, 2.0**-8]):
        sel[t, :] = s
    bf = ml_dtypes.bfloat16
    ckf = np.empty((NIN, N + 2 * NOUT + RPC), bf)
    ckf[:, N : N + NOUT] = Wj.T
    ckf[:, N + NOUT : N + 2 * NOUT] = Wi.T
    bf = ml_dtypes.bfloat16
    maps = []
    for c in range(NCORES):
        b, h = divmod(c, 2)
        r0 = RPC * h
        a = adj[b, 0, r0 : r0 + RPC, :]
        if h:
            ar = np.roll(a, -r0, axis=1)
            noder = np.roll(node[b], -r0, axis=1)
        else:
            ar = a
            noder = node[b]
        t0, t1, t2, t3 = _split_fp8_terms(ar.reshape(1, RPC * N))
        pk8 = np.empty((KP, CH * FREE + NOUT), f8)
        pk8[0, 0 : CH * FREE] = t0[0]
        pk8[1, 0 : CH * FREE] = t1[0]
        pk8[2, 0 : CH * FREE] = t2[0]
        pk8[3, 0 : CH * FREE] = t3[0]
        pk8[:, CH * FREE :] = sel
        m_ckf = ckf.copy()
        m_ckf[:, 0:N] = noder
        adiag = a[np.arange(RPC), r0 + np.arange(RPC)]
        m_ckf[:, N + 2 * NOUT :] = noder[:, 0:RPC] * adiag[None, :]
        af = ar.reshape(1, RPC * N).astype(bf)
        maps.append({"pk8": pk8, "ckf": m_ckf, "af": af})
    return maps


def kernel(**inputs):
    global last_results
    adj = np.asarray(inputs["adj"], dtype=np.float32)
    node = np.asarray(inputs["node"], dtype=np.float32)
    Wi = np.asarray(inputs["Wi"], dtype=np.float32)
    Wj = np.asarray(inputs["Wj"], dtype=np.float32)

    from concourse.bass_utils import run_bass_kernel_spmd

    nc = _build_nc()
    res = run_bass_kernel_spmd(nc, _in_maps(adj, node, Wi, Wj), list(range(NCORES)))
    last_results = res

    out = np.empty((B, NOUT, N, N), np.float32)
    for c in range(NCORES):
        b, h = divmod(c, 2)
        co = res.results[c]["out"].astype(np.float32).reshape(NOUT, RPC, N)
        if h:
            co = np.roll(co, RPC * h, axis=2)
        out[b, :, RPC * h : RPC * (h + 1), :] = co
    return out


# revision 36
# speedup vs baseline: 1.0883x; 1.0014x over previous
"""NodeConv kernel for 8 Trainium2 NeuronCores.

Reference computes, for adj [B,1,N,N], node [B,nin,N], Wi/Wj [nout,nin]:
    x  = node[:, :, None, :] * adj          # [B,nin,N,N]
    yi = einsum('oc,bcij->boij', Wi, x)
    yj = einsum('oc,bcij->boij', Wj, x)
    out = I * yi + (1-I) * yj

Because adj[b,i,j] does not depend on the contraction channel c, the
contraction factors out:
    off-diag: out[b,o,i,j] = adj[b,i,j] * (Wj @ node[b])[o,j]
    diag:     out[b,o,j,j] = adj[b,j,j] * (Wi @ node[b])[o,j]

Sharding: core c handles batch b=c//2, row half h=c%2 (128 rows). Odd
halves get their columns rolled by -128 on the host so the diagonal of
local row l sits at local column l on every core -> one SPMD program;
the host rolls the output back while gathering.

Design (measured down from the 72.8us f32 baseline to ~54us; tolerance
is 2e-2 and this path measures ~7.4e-3 max rel err):
  - OUTPUT IS BF16 (host upconverts to f32 while gathering): halves the
    HBM store traffic 16 MiB -> 8 MiB per core.  DMA floor ~21us.
  - adj is split host-side into 4 scaled e4m3 terms (reconstruction
    error ~4e-6) stored on psum partitions 0-3, chunk-major along the
    free dim, with a constant [4, NOUT] stationary holding the descale
    factors (1, 2^-4, 2^-8, 2^-8).  Each 512-col fp8 matmul broadcasts
    adj to all 128 output partitions in one pass (PE streams 1 col/cycle
    at the 1.2 GHz mid p-state regardless of dtype, so K and dtype are
    chosen purely to minimize the input load: 128 KB).
  - weights/node ship as bf16 so the u = Wj@node and dv = Wi@nodeD
    matmuls are single-pass (fp32 PE matmuls run as 2 half-speed
    passes).  nodeD is the diagonal node columns prescaled by adj's
    diagonal on the host, so dv IS the diagonal patch value - no
    diag-broadcast matmul needed.
  - per chunk (4 rows x 256 = 1024 cols, [128,1024] f32 psum tiles,
    4-buffer rotation so the PE->consumer round-trip never idles the
    pipeline): A-chunks are multiplied straight from PSUM on DVE
    (f32 x bf16 -> bf16, 1 elem/lane/cycle); B-chunks are copied
    PSUM f32 -> bf16 SBUF by ScalarE, then DVE multiplies all-bf16 in
    2x_1P mode (2 elem/lane/cycle).  The A/B mix balances ScalarE
    (~26us) against DVE (~27us) - the body floor.
  - diagonal patches (strided [128, RCH] writes of dv) run on the
    otherwise-idle GpSimd engine.
  - stores are issued from the Sync and GpSimd queues, group sizes
    tapered (small first groups start the DMA flow early, small last
    groups shorten the final drain); NODECONV_GORDER permutes group
    processing order.
  - GSET chunks (off by default) use GpSimd partition_broadcast of a
    host-rounded bf16 adj instead of the PE path; measured slower
    (~4.1us per chunk at 0.41 efficiency) but kept as a knob.
"""

import os

import numpy as np

NCORES = 8
B, N, NIN, NOUT = 4, 256, 128, 128
RPC = 128          # rows per core
RCH = int(os.environ.get("NODECONV_RCH", "4"))   # rows per chunk
CH = RPC // RCH    # chunks per core
FREE = RCH * N     # free elems per chunk
PSUM_BUFS = int(os.environ.get("NODECONV_PSUM_BUFS", "4" if RCH <= 4 else "2"))

# G-chunks: GpSimd partition_broadcast of host-rounded bf16 adj + DVE 2x_1P
# multiply — no PE/PSUM/ScalarE involvement.  Placed at the edges so the
# first stores fire early and the tail chunks are compute-ready early.
_GSET = {
    int(x)
    for x in os.environ.get("NODECONV_GSET", "").split(",")
    if x != ""
}
# C-chunks: ScalarE bf16 staging copy + GpSimd (standard-library) multiply —
# relieves the saturated DVE using GpSimd idle time.  Patches for these run
# in-order on GpSimd right after the multiply (no cross-engine semaphore).
_CSET = {
    int(x)
    for x in os.environ.get(
        "NODECONV_CSET", ""
    ).split(",")
    if x != ""
}
# A-chunks: multiplied directly from PSUM on DVE (1x); remaining (B) chunks
# go through a ScalarE bf16 staging copy + DVE 2x_1P multiply
_ASET = {
    int(x)
    for x in os.environ.get(
        "NODECONV_ASET",
        "0,4,8,12,16,20,24,27,30,31" if RCH == 4 else "0,5,10,15",
    ).split(",")
    if x != ""
}
_G = [
    int(x)
    for x in os.environ.get(
        "NODECONV_GROUPS",
        "1,1,2,2,4,4,4,4,4,2,2,2" if RCH == 4 else "2,2,2,2,2,2,2,1,1",
    ).split(",")
]
assert sum(_G) == CH
# processing order of the groups: the LAST-processed groups are the
# single-chunk ones (and the final chunk is A-type, the shortest
# mms->mul->patch->store chain), so almost no work remains after the last
# PE matmul; both tail chunks sit in the first-loaded pk8 piece
_GORDER = [
    int(x)
    for x in os.environ.get(
        "NODECONV_GORDER",
        "1,2,3,4,5,6,7,8,9,10,11,0" if RCH == 4 else ",".join(map(str, range(9))),
    ).split(",")
]
assert sorted(_GORDER) == list(range(len(_G)))
STAGE_BUFS = int(os.environ.get("NODECONV_STAGE_BUFS", "6"))
OUT_BUFS = int(os.environ.get("NODECONV_OUT_BUFS", "6"))
PATCH_ENG = os.environ.get("NODECONV_PATCH", "gp")  # gp | scalar | vector

KP = 4             # fp8 term partitions (t0..t3)

_cached = {}

last_results = None  # BassKernelResults of the most recent kernel() call


def _build_nc():
    key = (
        RCH,
        PSUM_BUFS,
        tuple(sorted(_ASET)),
        tuple(sorted(_CSET)),
        tuple(sorted(_GSET)),
        tuple(_G),
        tuple(_GORDER),
        STAGE_BUFS,
        OUT_BUFS,
        PATCH_ENG,
    )
    if key in _cached:
        return _cached[key]

    from contextlib import ExitStack

    import concourse.tile as tile
    from concourse import bacc, mybir

    f32 = mybir.dt.float32
    bf16 = mybir.dt.bfloat16
    fp8 = mybir.dt.float8e4

    nc = bacc.Bacc(
        "TRN2", target_bir_lowering=False, debug=False, num_devices=NCORES
    )

    # pk8: [4, CH*FREE + NOUT] fp8 — partition t holds scaled adj term t for
    # every chunk (chunk-major along the free dim), then the [4, NOUT]
    # stationary selector carrying the descale factors (1, 2^-4, 2^-8,
    # 2^-8).  Every matmul slices this at base partition 0 with the SAME
    # stationary, so no per-chunk selector blocks are needed.
    pk8 = nc.dram_tensor(
        "pk8", [KP, CH * FREE + NOUT], fp8, kind="ExternalInput"
    ).ap()
    # ckf: [128, 640] bf16 — node_r | WiT | WjT | nodeD, where nodeD is the
    # diagonal node columns prescaled by adj's diagonal on the host, so
    # Wi @ nodeD directly yields the diagonal patch values dv.  bf16 keeps
    # the u/dv matmuls single-pass (fp32 PE matmuls run as 2 half-speed
    # passes) and halves the critical first input load.
    ckf = nc.dram_tensor(
        "ckf", [NIN, N + 2 * NOUT + RPC], bf16, kind="ExternalInput"
    ).ap()
    # af: chunk p's eight adj rows flattened on partition 0 (bf16) — the
    # source for GpSimd partition_broadcast on G-chunks
    af = nc.dram_tensor("af", [1, RPC * N], bf16, kind="ExternalInput").ap()
    out = nc.dram_tensor("out", [NOUT, RPC * N], bf16, kind="ExternalOutput").ap()

    with tile.TileContext(nc) as tc, ExitStack() as ctx:
        const = ctx.enter_context(tc.tile_pool(name="const", bufs=1))
        psum = ctx.enter_context(tc.tile_pool(name="psum", bufs=PSUM_BUFS, space="PSUM"))
        outp = ctx.enter_context(tc.tile_pool(name="outp", bufs=OUT_BUFS))
        stage = ctx.enter_context(tc.tile_pool(name="stage", bufs=STAGE_BUFS))
        bcp = ctx.enter_context(tc.tile_pool(name="bcp", bufs=len(_GSET) or 1))

        # Input loads are split across the sync and gpsimd queues: the
        # 4-partition pk8 tensor transfers slowly per descriptor, so each
        # queue carries half, and ckf's u-critical piece (node|WjT) plus the
        # matmul selector go out first on sync.
        ckf_sb = const.tile([NIN, N + 2 * NOUT + RPC], bf16)
        pk8_sb = const.tile([KP, CH * FREE + NOUT], fp8)
        # Pieces land in processing order (GORDER runs chunks 0,1,28..31,
        # 2,3,... first): the tiny selector + chunks 0-1 go first on sync so
        # the first matmul fires ~3us earlier; ckf halves lead each queue's
        # bulk so the u/dv chain and the early-processed tail chunks are
        # never the gating load.
        def dma(eng, lo, hi):
            eng.dma_start(out=pk8_sb[:, lo:hi], in_=pk8[:, lo:hi])

        dma(nc.sync, CH * FREE, CH * FREE + NOUT)          # selector
        dma(nc.sync, 0, 2 * FREE)                          # chunks 0-1
        nc.sync.dma_start(out=ckf_sb[:, 0 : N + NOUT], in_=ckf[:, 0 : N + NOUT])
        nc.gpsimd.dma_start(
            out=ckf_sb[:, N + NOUT :], in_=ckf[:, N + NOUT :]
        )
        dma(nc.gpsimd, 2 * FREE, 16 * FREE)                # chunks 2-15
        dma(nc.sync, 16 * FREE, 28 * FREE)                 # chunks 16-27
        dma(nc.gpsimd, 28 * FREE, 32 * FREE)               # chunks 28-31
        if _GSET:
            af_sb = const.tile([1, RPC * N], bf16)
            nc.gpsimd.dma_start(out=af_sb[:], in_=af)

        node_sb = ckf_sb[:, 0:N]
        wjt_sb = ckf_sb[:, N : N + NOUT]
        wit_sb = ckf_sb[:, N + NOUT : N + 2 * NOUT]
        noded_sb = ckf_sb[:, N + 2 * NOUT : N + 2 * NOUT + RPC]
        sel_v = pk8_sb[:, CH * FREE : CH * FREE + NOUT]
        pk_v = pk8_sb[:, 0 : CH * FREE]

        # u = Wj @ node_r -> [nout, N], and (later) dv = Wi @ nodeD.  Both
        # live in one psum tile so the chunk psum tiles keep alternating
        # between the pool's two buffers.
        u_bf = const.tile([NOUT, N], bf16)
        dv_bf = const.tile([NOUT, RPC], bf16)
        _uv_emitted = [False]

        def emit_uv():
            # u = Wj @ node_r and dv = Wi @ nodeD, emitted right after the
            # first chunk's matmuls so the pk8-gated broadcast work heads
            # the PE queue; the dv copy precedes every patch in program
            # order (patches read dv_bf)
            ps_uv = psum.tile([NOUT, N + RPC], f32, tag="mm")
            nc.tensor.matmul(
                ps_uv[:, 0:N], lhsT=wjt_sb, rhs=node_sb, start=True, stop=True
            )
            nc.scalar.copy(u_bf[:], ps_uv[:, 0:N])
            nc.tensor.matmul(
                ps_uv[:, N : N + RPC],
                lhsT=wit_sb,
                rhs=noded_sb,
                start=True,
                stop=True,
            )
            nc.scalar.copy(dv_bf[:], ps_uv[:, N : N + RPC])
            _uv_emitted[0] = True

        # u replicated RCH (and 2*RCH for fused B-pairs) times along the
        # free dim via stride-0 views
        u16_rep = u_bf[:].unsqueeze(1).broadcast_to([NOUT, RCH, N])
        u16_rep2 = u_bf[:].unsqueeze(1).broadcast_to([NOUT, 2 * RCH, N])

        patch_eng = {
            "gp": nc.gpsimd,
            "scalar": nc.scalar,
            "vector": nc.vector,
        }[PATCH_ENG]

        gstart = [0]
        for gsz in _G:
            gstart.append(gstart[-1] + gsz)
        for oi, gi in enumerate(_GORDER):
            gsz = _G[gi]
            o_sb = outp.tile([NOUT, gsz * FREE], bf16, tag="osb")
            p0 = gstart[gi]
            # adjacent B-chunks within a group share one stage tile and a
            # single fused DVE multiply over both (fewer DVE instructions)
            bpair = {}
            run = []
            for g in range(gsz):
                q_ = p0 + g
                if q_ not in _ASET and q_ not in _GSET and q_ not in _CSET:
                    run.append(g)
                    if len(run) == 2:
                        st2 = stage.tile([NOUT, 2 * FREE], bf16, tag="st")
                        bpair[run[0]] = (st2, 0, None)
                        bpair[run[1]] = (st2, 1, run[0])
                        run = []
                else:
                    run = []
            def emit_patch(eng_, g_, p_):
                # diagonal of local row l sits at free offset RCH*p + k*257
                eng_.tensor_scalar_mul(
                    o_sb[
                        :,
                        g_ * FREE + RCH * p_ : g_ * FREE
                        + RCH * p_
                        + (RCH - 1) * (N + 1)
                        + 1 : N + 1,
                    ],
                    dv_bf[:, RCH * p_ : RCH * (p_ + 1)],
                    1.0,
                )

            for g in range(gsz):
                p = p0 + g
                defer_patch = False
                o_view = o_sb[:, g * FREE : (g + 1) * FREE].rearrange(
                    "p (k j) -> p k j", k=RCH
                )
                if p in _GSET:
                    bc = bcp.tile([NOUT, FREE], bf16, tag="bc")
                    nc.gpsimd.partition_broadcast(
                        bc[:], af_sb[:, FREE * p : FREE * (p + 1)]
                    )
                    nc.vector.tensor_mul(
                        o_view, bc[:].rearrange("p (k j) -> p k j", k=RCH), u16_rep
                    )
                    peng = nc.vector
                else:
                    ps_b = psum.tile([NOUT, FREE], f32, tag="mm")
                    for q in range(FREE // 512):
                        nc.tensor.matmul(
                            ps_b[:, 512 * q : 512 * (q + 1)],
                            lhsT=sel_v,
                            rhs=pk_v[
                                :, FREE * p + 512 * q : FREE * p + 512 * (q + 1)
                            ],
                            start=True,
                            stop=True,
                        )
                    if not _uv_emitted[0]:
                        emit_uv()
                    if p in _ASET:
                        nc.vector.tensor_mul(
                            o_view,
                            ps_b[:].rearrange("p (k j) -> p k j", k=RCH),
                            u16_rep,
                        )
                    elif p in _CSET:
                        st = stage.tile([NOUT, FREE], bf16, tag="st")
                        nc.scalar.copy(st[:], ps_b[:])
                        nc.gpsimd.tensor_mul(
                            o_view,
                            st[:].rearrange("p (k j) -> p k j", k=RCH),
                            u16_rep,
                        )
                    elif g in bpair:
                        st2, half, first_g = bpair[g]
                        nc.scalar.copy(
                            st2[:, half * FREE : (half + 1) * FREE], ps_b[:]
                        )
                        if half == 1:
                            # both halves staged: one fused 2-chunk multiply,
                            # then the deferred patch of the pair's first
                            # chunk (it must come after the fused multiply)
                            nc.vector.tensor_mul(
                                o_sb[
                                    :, first_g * FREE : (first_g + 2) * FREE
                                ].rearrange("p (k j) -> p k j", k=2 * RCH),
                                st2[:].rearrange("p (k j) -> p k j", k=2 * RCH),
                                u16_rep2,
                            )
                            emit_patch(patch_eng, first_g, p0 + first_g)
                        else:
                            defer_patch = True
                    else:
                        st = stage.tile([NOUT, FREE], bf16, tag="st")
                        nc.scalar.copy(st[:], ps_b[:])
                        nc.vector.tensor_mul(
                            o_view,
                            st[:].rearrange("p (k j) -> p k j", k=RCH),
                            u16_rep,
                        )
                    peng = patch_eng
                if not defer_patch:
                    emit_patch(peng, g, p)
            eng = nc.sync if oi % 2 == 0 else nc.gpsimd
            eng.dma_start(
                out=out[:, FREE * p0 : FREE * (p0 + gsz)], in_=o_sb[:]
            )

    nc.compile()
    _cached[key] = nc
    return nc


def _split_fp8_terms(x):
    """Split fp32 array (values in [0,1)) into 4 e4m3 terms with scales
    (1, 2^4, 2^8, 2^8) whose descaled f32 sum reconstructs x to ~4e-6."""
    import ml_dtypes

    f8 = ml_dtypes.float8_e4m3
    t0 = x.astype(f8)
    r = x - t0.astype(np.float32)
    t1 = (r * 16.0).astype(f8)
    r = r - t1.astype(np.float32) / 16.0
    t2 = (r * 256.0).astype(f8)
    r = r - t2.astype(np.float32) / 256.0
    t3 = (r * 256.0).astype(f8)
    return t0, t1, t2, t3


def _in_maps(adj, node, Wi, Wj):
    import ml_dtypes

    f8 = ml_dtypes.float8_e4m3
    sel = np.empty((KP, NOUT), f8)
    for t, s in enumerate([1.0, 2.0**-4, 2.0**-8, 2.0**-8]):
        sel[t, :] = s
    bf = ml_dtypes.bfloat16
    ckf = np.empty((NIN, N + 2 * NOUT + RPC), bf)
    ckf[:, N : N + NOUT] = Wj.T
    ckf[:, N + NOUT : N + 2 * NOUT] = Wi.T
    bf = ml_dtypes.bfloat16
    maps = []
    for c in range(NCORES):
        b, h = divmod(c, 2)
        r0 = RPC * h
        a = adj[b, 0, r0 : r0 + RPC, :]
        if h:
            ar = np.roll(a, -r0, axis=1)
            noder = np.roll(node[b], -r0, axis=1)
        else:
            ar = a
            noder = node[b]
        t0, t1, t2, t3 = _split_fp8_terms(ar.reshape(1, RPC * N))
        pk8 = np.empty((KP, CH * FREE + NOUT), f8)
        pk8[0, 0 : CH * FREE] = t0[0]
        pk8[1, 0 : CH * FREE] = t1[0]
        pk8[2, 0 : CH * FREE] = t2[0]
        pk8[3, 0 : CH * FREE] = t3[0]
        pk8[:, CH * FREE :] = sel
        m_ckf = ckf.copy()
        m_ckf[:, 0:N] = noder
        adiag = a[np.arange(RPC), r0 + np.arange(RPC)]
        m_ckf[:, N + 2 * NOUT :] = noder[:, 0:RPC] * adiag[None, :]
        af = ar.reshape(1, RPC * N).astype(bf)
        maps.append({"pk8": pk8, "ckf": m_ckf, "af": af})
    return maps


def kernel(**inputs):
    global last_results
    adj = np.asarray(inputs["adj"], dtype=np.float32)
    node = np.asarray(inputs["node"], dtype=np.float32)
    Wi = np.asarray(inputs["Wi"], dtype=np.float32)
    Wj = np.asarray(inputs["Wj"], dtype=np.float32)

    from concourse.bass_utils import run_bass_kernel_spmd

    nc = _build_nc()
    res = run_bass_kernel_spmd(nc, _in_maps(adj, node, Wi, Wj), list(range(NCORES)))
    last_results = res

    out = np.empty((B, NOUT, N, N), np.float32)
    for c in range(NCORES):
        b, h = divmod(c, 2)
        co = res.results[c]["out"].astype(np.float32).reshape(NOUT, RPC, N)
        if h:
            co = np.roll(co, RPC * h, axis=2)
        out[b, :, RPC * h : RPC * (h + 1), :] = co
    return out


# revision 37
# speedup vs baseline: 1.1546x; 1.0609x over previous
"""NodeConv kernel for 8 Trainium2 NeuronCores.

Reference computes, for adj [B,1,N,N], node [B,nin,N], Wi/Wj [nout,nin]:
    x  = node[:, :, None, :] * adj          # [B,nin,N,N]
    yi = einsum('oc,bcij->boij', Wi, x)
    yj = einsum('oc,bcij->boij', Wj, x)
    out = I * yi + (1-I) * yj

Because adj[b,i,j] does not depend on the contraction channel c, the
contraction factors out:
    off-diag: out[b,o,i,j] = adj[b,i,j] * (Wj @ node[b])[o,j]
    diag:     out[b,o,j,j] = adj[b,j,j] * (Wi @ node[b])[o,j]

Sharding: core c handles batch b=c//2, row half h=c%2 (128 rows). Odd
halves get their columns rolled by -128 on the host so the diagonal of
local row l sits at local column l on every core -> one SPMD program;
the host rolls the output back while gathering.

Design (measured down from the 72.8us f32 baseline to ~54us; tolerance
is 2e-2 and this path measures ~7.4e-3 max rel err):
  - OUTPUT IS BF16 (host upconverts to f32 while gathering): halves the
    HBM store traffic 16 MiB -> 8 MiB per core.  DMA floor ~21us.
  - adj is split host-side into 4 scaled e4m3 terms (reconstruction
    error ~4e-6) stored on psum partitions 0-3, chunk-major along the
    free dim, with a constant [4, NOUT] stationary holding the descale
    factors (1, 2^-4, 2^-8, 2^-8).  Each 512-col fp8 matmul broadcasts
    adj to all 128 output partitions in one pass (PE streams 1 col/cycle
    at the 1.2 GHz mid p-state regardless of dtype, so K and dtype are
    chosen purely to minimize the input load: 128 KB).
  - weights/node ship as bf16 so the u = Wj@node and dv = Wi@nodeD
    matmuls are single-pass (fp32 PE matmuls run as 2 half-speed
    passes).  nodeD is the diagonal node columns prescaled by adj's
    diagonal on the host, so dv IS the diagonal patch value - no
    diag-broadcast matmul needed.
  - per chunk (4 rows x 256 = 1024 cols, [128,1024] f32 psum tiles,
    4-buffer rotation so the PE->consumer round-trip never idles the
    pipeline): A-chunks are multiplied straight from PSUM on DVE
    (f32 x bf16 -> bf16, 1 elem/lane/cycle); B-chunks are copied
    PSUM f32 -> bf16 SBUF by ScalarE, then DVE multiplies all-bf16 in
    2x_1P mode (2 elem/lane/cycle).  The A/B mix balances ScalarE
    (~26us) against DVE (~27us) - the body floor.
  - diagonal patches (strided [128, RCH] writes of dv) run on the
    otherwise-idle GpSimd engine.
  - stores are issued from the Sync and GpSimd queues, group sizes
    tapered (small first groups start the DMA flow early, small last
    groups shorten the final drain); NODECONV_GORDER permutes group
    processing order.
  - GSET chunks (off by default) use GpSimd partition_broadcast of a
    host-rounded bf16 adj instead of the PE path; measured slower
    (~4.1us per chunk at 0.41 efficiency) but kept as a knob.
"""

import os

import numpy as np

NCORES = 8
B, N, NIN, NOUT = 4, 256, 128, 128
RPC = 128          # rows per core
RCH = int(os.environ.get("NODECONV_RCH", "4"))   # rows per chunk
CH = RPC // RCH    # chunks per core
FREE = RCH * N     # free elems per chunk
PSUM_BUFS = int(os.environ.get("NODECONV_PSUM_BUFS", "4" if RCH <= 4 else "2"))

# G-chunks: GpSimd partition_broadcast of host-rounded bf16 adj + DVE 2x_1P
# multiply — no PE/PSUM/ScalarE involvement.  Placed at the edges so the
# first stores fire early and the tail chunks are compute-ready early.
_GSET = {
    int(x)
    for x in os.environ.get("NODECONV_GSET", "").split(",")
    if x != ""
}
# C-chunks: ScalarE bf16 staging copy + GpSimd (standard-library) multiply —
# relieves the saturated DVE using GpSimd idle time.  Patches for these run
# in-order on GpSimd right after the multiply (no cross-engine semaphore).
_CSET = {
    int(x)
    for x in os.environ.get(
        "NODECONV_CSET", ""
    ).split(",")
    if x != ""
}
# A-chunks: multiplied directly from PSUM on DVE (1x); remaining (B) chunks
# go through a ScalarE bf16 staging copy + DVE 2x_1P multiply
_ASET = {
    int(x)
    for x in os.environ.get(
        "NODECONV_ASET",
        "0,4,8,12,16,20,24,27,30,31" if RCH == 4 else "0,5,10,15",
    ).split(",")
    if x != ""
}
_G = [
    int(x)
    for x in os.environ.get(
        "NODECONV_GROUPS",
        "1,1,2,2,4,4,4,4,4,2,2,2" if RCH == 4 else "2,2,2,2,2,2,2,1,1",
    ).split(",")
]
assert sum(_G) == CH
# processing order of the groups: singleton groups first (early store
# flow), DRAM-tail groups early, small groups last; measured best
_GORDER = [
    int(x)
    for x in os.environ.get(
        "NODECONV_GORDER",
        "0,1,10,11,2,3,4,5,6,7,8,9" if RCH == 4 else ",".join(map(str, range(9))),
    ).split(",")
]
assert sorted(_GORDER) == list(range(len(_G)))
STAGE_BUFS = int(os.environ.get("NODECONV_STAGE_BUFS", "6"))
OUT_BUFS = int(os.environ.get("NODECONV_OUT_BUFS", "6"))
PATCH_ENG = os.environ.get("NODECONV_PATCH", "gp")  # gp | scalar | vector

KP = 4             # fp8 term partitions (t0..t3)

_cached = {}

last_results = None  # BassKernelResults of the most recent kernel() call


def _build_nc():
    key = (
        RCH,
        PSUM_BUFS,
        tuple(sorted(_ASET)),
        tuple(sorted(_CSET)),
        tuple(sorted(_GSET)),
        tuple(_G),
        tuple(_GORDER),
        STAGE_BUFS,
        OUT_BUFS,
        PATCH_ENG,
    )
    if key in _cached:
        return _cached[key]

    from contextlib import ExitStack

    import concourse.tile as tile
    from concourse import bacc, mybir

    f32 = mybir.dt.float32
    bf16 = mybir.dt.bfloat16
    fp8 = mybir.dt.float8e4

    nc = bacc.Bacc(
        "TRN2", target_bir_lowering=False, debug=False, num_devices=NCORES
    )

    # pk8: [4, CH*FREE + NOUT] fp8 — partition t holds scaled adj term t for
    # every chunk (chunk-major along the free dim), then the [4, NOUT]
    # stationary selector carrying the descale factors (1, 2^-4, 2^-8,
    # 2^-8).  Every matmul slices this at base partition 0 with the SAME
    # stationary, so no per-chunk selector blocks are needed.
    pk8 = nc.dram_tensor(
        "pk8", [KP, CH * FREE + NOUT], fp8, kind="ExternalInput"
    ).ap()
    # ckf: [128, 640] bf16 — node_r | WiT | WjT | nodeD, where nodeD is the
    # diagonal node columns prescaled by adj's diagonal on the host, so
    # Wi @ nodeD directly yields the diagonal patch values dv.  bf16 keeps
    # the u/dv matmuls single-pass (fp32 PE matmuls run as 2 half-speed
    # passes) and halves the critical first input load.
    ckf = nc.dram_tensor(
        "ckf", [NIN, N + 2 * NOUT + RPC], bf16, kind="ExternalInput"
    ).ap()
    # af: chunk p's eight adj rows flattened on partition 0 (bf16) — the
    # source for GpSimd partition_broadcast on G-chunks
    af = nc.dram_tensor("af", [1, RPC * N], bf16, kind="ExternalInput").ap()
    out = nc.dram_tensor("out", [NOUT, RPC * N], bf16, kind="ExternalOutput").ap()

    with tile.TileContext(nc) as tc, ExitStack() as ctx:
        const = ctx.enter_context(tc.tile_pool(name="const", bufs=1))
        psum = ctx.enter_context(tc.tile_pool(name="psum", bufs=PSUM_BUFS, space="PSUM"))
        outp = ctx.enter_context(tc.tile_pool(name="outp", bufs=OUT_BUFS))
        stage = ctx.enter_context(tc.tile_pool(name="stage", bufs=STAGE_BUFS))
        bcp = ctx.enter_context(tc.tile_pool(name="bcp", bufs=len(_GSET) or 1))

        # Input loads are split across the sync and gpsimd queues: the
        # 4-partition pk8 tensor transfers slowly per descriptor, so each
        # queue carries half, and ckf's u-critical piece (node|WjT) plus the
        # matmul selector go out first on sync.
        ckf_sb = const.tile([NIN, N + 2 * NOUT + RPC], bf16)
        pk8_sb = const.tile([KP, CH * FREE + NOUT], fp8)
        # Pieces land in processing order (GORDER runs chunks 0,1,28..31,
        # 2,3,... first): the tiny selector + chunks 0-1 go first on sync so
        # the first matmul fires ~3us earlier; ckf halves lead each queue's
        # bulk so the u/dv chain and the early-processed tail chunks are
        # never the gating load.
        def dma(eng, lo, hi):
            eng.dma_start(out=pk8_sb[:, lo:hi], in_=pk8[:, lo:hi])

        dma(nc.sync, CH * FREE, CH * FREE + NOUT)          # selector
        dma(nc.sync, 0, 2 * FREE)                          # chunks 0-1
        nc.sync.dma_start(out=ckf_sb[:, 0 : N + NOUT], in_=ckf[:, 0 : N + NOUT])
        nc.gpsimd.dma_start(
            out=ckf_sb[:, N + NOUT :], in_=ckf[:, N + NOUT :]
        )
        dma(nc.gpsimd, 28 * FREE, 32 * FREE)               # chunks 28-31
        dma(nc.gpsimd, 2 * FREE, 16 * FREE)                # chunks 2-15
        dma(nc.sync, 16 * FREE, 28 * FREE)                 # chunks 16-27
        if _GSET:
            af_sb = const.tile([1, RPC * N], bf16)
            nc.gpsimd.dma_start(out=af_sb[:], in_=af)

        node_sb = ckf_sb[:, 0:N]
        wjt_sb = ckf_sb[:, N : N + NOUT]
        wit_sb = ckf_sb[:, N + NOUT : N + 2 * NOUT]
        noded_sb = ckf_sb[:, N + 2 * NOUT : N + 2 * NOUT + RPC]
        sel_v = pk8_sb[:, CH * FREE : CH * FREE + NOUT]
        pk_v = pk8_sb[:, 0 : CH * FREE]

        # u = Wj @ node_r -> [nout, N], and (later) dv = Wi @ nodeD.  Both
        # live in one psum tile so the chunk psum tiles keep alternating
        # between the pool's two buffers.
        u_bf = const.tile([NOUT, N], bf16)
        dv_bf = const.tile([NOUT, RPC], bf16)
        _uv_emitted = [False]

        def emit_uv():
            # u = Wj @ node_r and dv = Wi @ nodeD, emitted right after the
            # first chunk's matmuls so the pk8-gated broadcast work heads
            # the PE queue; the dv copy precedes every patch in program
            # order (patches read dv_bf)
            ps_uv = psum.tile([NOUT, N + RPC], f32, tag="mm")
            nc.tensor.matmul(
                ps_uv[:, 0:N], lhsT=wjt_sb, rhs=node_sb, start=True, stop=True
            )
            nc.scalar.copy(u_bf[:], ps_uv[:, 0:N])
            nc.tensor.matmul(
                ps_uv[:, N : N + RPC],
                lhsT=wit_sb,
                rhs=noded_sb,
                start=True,
                stop=True,
            )
            nc.scalar.copy(dv_bf[:], ps_uv[:, N : N + RPC])
            _uv_emitted[0] = True

        # u replicated RCH (and 2*RCH for fused B-pairs) times along the
        # free dim via stride-0 views
        u16_rep = u_bf[:].unsqueeze(1).broadcast_to([NOUT, RCH, N])
        u16_rep2 = u_bf[:].unsqueeze(1).broadcast_to([NOUT, 2 * RCH, N])

        patch_eng = {
            "gp": nc.gpsimd,
            "scalar": nc.scalar,
            "vector": nc.vector,
        }[PATCH_ENG]

        gstart = [0]
        for gsz in _G:
            gstart.append(gstart[-1] + gsz)
        for oi, gi in enumerate(_GORDER):
            gsz = _G[gi]
            o_sb = outp.tile([NOUT, gsz * FREE], bf16, tag="osb")
            p0 = gstart[gi]
            # adjacent B-chunks within a group share one stage tile and a
            # single fused DVE multiply over both (fewer DVE instructions)
            bpair = {}
            run = []
            for g in range(gsz):
                q_ = p0 + g
                if q_ not in _ASET and q_ not in _GSET and q_ not in _CSET:
                    run.append(g)
                    if len(run) == 2:
                        st2 = stage.tile([NOUT, 2 * FREE], bf16, tag="st")
                        bpair[run[0]] = (st2, 0, None)
                        bpair[run[1]] = (st2, 1, run[0])
                        run = []
                else:
                    run = []
            def emit_patch(eng_, g_, p_):
                # diagonal of local row l sits at free offset RCH*p + k*257
                eng_.tensor_scalar_mul(
                    o_sb[
                        :,
                        g_ * FREE + RCH * p_ : g_ * FREE
                        + RCH * p_
                        + (RCH - 1) * (N + 1)
                        + 1 : N + 1,
                    ],
                    dv_bf[:, RCH * p_ : RCH * (p_ + 1)],
                    1.0,
                )

            for g in range(gsz):
                p = p0 + g
                defer_patch = False
                o_view = o_sb[:, g * FREE : (g + 1) * FREE].rearrange(
                    "p (k j) -> p k j", k=RCH
                )
                if p in _GSET:
                    bc = bcp.tile([NOUT, FREE], bf16, tag="bc")
                    nc.gpsimd.partition_broadcast(
                        bc[:], af_sb[:, FREE * p : FREE * (p + 1)]
                    )
                    nc.vector.tensor_mul(
                        o_view, bc[:].rearrange("p (k j) -> p k j", k=RCH), u16_rep
                    )
                    peng = nc.vector
                else:
                    ps_b = psum.tile([NOUT, FREE], f32, tag="mm")
                    for q in range(FREE // 512):
                        nc.tensor.matmul(
                            ps_b[:, 512 * q : 512 * (q + 1)],
                            lhsT=sel_v,
                            rhs=pk_v[
                                :, FREE * p + 512 * q : FREE * p + 512 * (q + 1)
                            ],
                            start=True,
                            stop=True,
                        )
                    if not _uv_emitted[0]:
                        emit_uv()
                    if p in _ASET:
                        nc.vector.tensor_mul(
                            o_view,
                            ps_b[:].rearrange("p (k j) -> p k j", k=RCH),
                            u16_rep,
                        )
                    elif p in _CSET:
                        st = stage.tile([NOUT, FREE], bf16, tag="st")
                        nc.scalar.copy(st[:], ps_b[:])
                        nc.gpsimd.tensor_mul(
                            o_view,
                            st[:].rearrange("p (k j) -> p k j", k=RCH),
                            u16_rep,
                        )
                    elif g in bpair:
                        st2, half, first_g = bpair[g]
                        nc.scalar.copy(
                            st2[:, half * FREE : (half + 1) * FREE], ps_b[:]
                        )
                        if half == 1:
                            # both halves staged: one fused 2-chunk multiply,
                            # then the deferred patch of the pair's first
                            # chunk (it must come after the fused multiply)
                            nc.vector.tensor_mul(
                                o_sb[
                                    :, first_g * FREE : (first_g + 2) * FREE
                                ].rearrange("p (k j) -> p k j", k=2 * RCH),
                                st2[:].rearrange("p (k j) -> p k j", k=2 * RCH),
                                u16_rep2,
                            )
                            emit_patch(patch_eng, first_g, p0 + first_g)
                        else:
                            defer_patch = True
                    else:
                        st = stage.tile([NOUT, FREE], bf16, tag="st")
                        nc.scalar.copy(st[:], ps_b[:])
                        nc.vector.tensor_mul(
                            o_view,
                            st[:].rearrange("p (k j) -> p k j", k=RCH),
                            u16_rep,
                        )
                    peng = patch_eng
                if not defer_patch:
                    emit_patch(peng, g, p)
            eng = nc.sync if oi % 2 == 0 else nc.gpsimd
            eng.dma_start(
                out=out[:, FREE * p0 : FREE * (p0 + gsz)], in_=o_sb[:]
            )

    nc.compile()
    _cached[key] = nc
    return nc


def _split_fp8_terms(x):
    """Split fp32 array (values in [0,1)) into 4 e4m3 terms with scales
    (1, 2^4, 2^8, 2^8) whose descaled f32 sum reconstructs x to ~4e-6."""
    import ml_dtypes

    f8 = ml_dtypes.float8_e4m3
    t0 = x.astype(f8)
    r = x - t0.astype(np.float32)
    t1 = (r * 16.0).astype(f8)
    r = r - t1.astype(np.float32) / 16.0
    t2 = (r * 256.0).astype(f8)
    r = r - t2.astype(np.float32) / 256.0
    t3 = (r * 256.0).astype(f8)
    return t0, t1, t2, t3


def _in_maps(adj, node, Wi, Wj):
    import ml_dtypes

    f8 = ml_dtypes.float8_e4m3
    sel = np.empty((KP, NOUT), f8)
    for t, s in enumerate([1.0, 2.0**-4, 2.0**-8, 2.0**-8]):
        sel[t, :] = s
    bf = ml_dtypes.bfloat16
    ckf = np.empty((NIN, N + 2 * NOUT + RPC), bf)
    ckf[:, N : N + NOUT] = Wj.T
    ckf[:, N + NOUT : N + 2 * NOUT] = Wi.T
    bf = ml_dtypes.bfloat16
    maps = []
    for c in range(NCORES):
        b, h = divmod(c, 2)
        r0 = RPC * h
        a = adj[b, 0, r0 : r0 + RPC, :]
        if h:
            ar = np.roll(a, -r0, axis=1)
            noder = np.roll(node[b], -r0, axis=1)
        else:
            ar = a
            noder = node[b]
        t0, t1, t2, t3 = _split_fp8_terms(ar.reshape(1, RPC * N))
        pk8 = np.empty((KP, CH * FREE + NOUT), f8)
        pk8[0, 0 : CH * FREE] = t0[0]
        pk8[1, 0 : CH * FREE] = t1[0]
        pk8[2, 0 : CH * FREE] = t2[0]
        pk8[3, 0 : CH * FREE] = t3[0]
        pk8[:, CH * FREE :] = sel
        m_ckf = ckf.copy()
        m_ckf[:, 0:N] = noder
        adiag = a[np.arange(RPC), r0 + np.arange(RPC)]
        m_ckf[:, N + 2 * NOUT :] = noder[:, 0:RPC] * adiag[None, :]
        af = ar.reshape(1, RPC * N).astype(bf)
        maps.append({"pk8": pk8, "ckf": m_ckf, "af": af})
    return maps


def kernel(**inputs):
    global last_results
    adj = np.asarray(inputs["adj"], dtype=np.float32)
    node = np.asarray(inputs["node"], dtype=np.float32)
    Wi = np.asarray(inputs["Wi"], dtype=np.float32)
    Wj = np.asarray(inputs["Wj"], dtype=np.float32)

    from concourse.bass_utils import run_bass_kernel_spmd

    nc = _build_nc()
    res = run_bass_kernel_spmd(nc, _in_maps(adj, node, Wi, Wj), list(range(NCORES)))
    last_results = res

    out = np.empty((B, NOUT, N, N), np.float32)
    for c in range(NCORES):
        b, h = divmod(c, 2)
        co = res.results[c]["out"].astype(np.float32).reshape(NOUT, RPC, N)
        if h:
            co = np.roll(co, RPC * h, axis=2)
        out[b, :, RPC * h : RPC * (h + 1), :] = co
    return out
